# revision 26
# baseline (speedup 1.0000x reference)
"""Multi-head attention with RoPE on 8 Trainium2 NeuronCores.

Problem: x[2,2048,1024] -> MHA(16 heads, hd=64, NeoX RoPE, non-causal) -> out.

Sharding: tensor-parallel over heads. Each core owns 2 heads:
  - computes q^T,k^T (RoPE'd) and v for its heads over the full sequence
    (weights column-sliced on host; x replicated),
  - flash-style attention entirely on-chip with *transposed* scores
    [s_k, s_q] so the softmax denominator comes from a fused ones-column
    in V (no P transpose, no row-max pass; a constant bias inside the exp
    activation keeps the range safe),
  - one small AllToAll redistributes attention outputs so each core holds
    all 1024 attn dims for its 512-row output shard,
  - local Wo matmul produces the shard; host concatenates shards.

All matmuls run in bf16 (fp32 PSUM accumulation); rel-err tolerance is 2e-2.
"""

import sys

sys.path.insert(0, "/opt/trn_rl_repo")

import numpy as np  # noqa: E402

import concourse.bass as bass  # noqa: E402
import concourse.mybir as mybir  # noqa: E402
import concourse.tile as tile  # noqa: E402
from concourse.bass_utils import run_bass_kernel_spmd  # noqa: E402

N_CORES = 8
D = 1024
H = 16
HD = 64
HL = H // N_CORES  # local heads per core
DL = HL * HD  # 128 local attn dims
EXP_SCALE = 0.125  # 1/sqrt(hd)
EXP_BIAS = -24.0  # exp(s/8 - 24): cancels in softmax, keeps fp32 range safe
GMAX = 3  # scores-psum banks per (head, kt-group); 2*(GMAX banks)+2 PV <= 8

F32 = mybir.dt.float32
BF16 = mybir.dt.bfloat16


def _kt_groups(kt):
    """Split kt score-tiles into groups of <=GMAX (wider exp instructions)."""
    groups = []
    k0 = 0
    while k0 < kt:
        n3 = (kt - k0) // GMAX
        g = GMAX if n3 > 0 and (kt - k0) % GMAX != 1 else min(GMAX - 1, kt - k0)
        if (kt - k0) % GMAX == 0:
            g = GMAX
        groups.append((k0, g))
        k0 += g
    return groups


def _perm_matrix():
    """lhsT for the rotate_half matmul: qrot^T = lhsT.T @ q^T.

    Per head block at offset o: rot(q)[i] = -q[i+32] for i<32,
    rot(q)[i] = q[i-32] for 32<=i<64.
    """
    mt = np.zeros((DL, DL), dtype=np.float32)
    for o in (0, HD):
        for r in range(HD // 2):
            mt[o + r, o + r + HD // 2] = -1.0
            mt[o + r + HD // 2, o + r] = 1.0
    return np.ascontiguousarray(mt.T)


def split_excess_waits(nc, max_waits=1):
    """This container's walrus rejects >1 semaphore wait per instruction;
    split excess waits onto NoOp carriers on the same engine."""
    for bb in nc.m.functions[0].blocks:
        insts = bb.instructions
        idx = 0
        while idx < len(insts):
            ins = insts[idx]
            si = ins.sync_info
            if si is not None and si.on_wait and len(si.on_wait) > max_waits:
                ow = list(si.on_wait)
                si.on_wait = ow[-max_waits:]
                extra = ow[:-max_waits]
                k = 0
                while extra:
                    chunk, extra = extra[:max_waits], extra[max_waits:]
                    c = mybir.InstNoOp(name=f"{ins.name}-ws{k}", ins=[], outs=[])
                    c.engine = ins.engine
                    c.sync_info = mybir.SyncInfo(on_wait=chunk, on_update=[])
                    nc.register_instruction(c)
                    insts.insert(idx, c)
                    idx += 1
                    k += 1
            idx += 1


def build_nc(b=2, s=2048, chunk=512, pt_bufs=12, debug=False):
    kt = s // 128  # score tiles along s_k per batch
    nch = s // chunk  # s_q chunks per batch
    shard = b * s // N_CORES  # output rows per core
    groups = _kt_groups(kt)
    dt8 = D // 128  # contraction tiles for projections

    nc = bass.Bass()
    x = nc.declare_dram_parameter("x", [b, s, D], F32, isOutput=False)
    cosp = nc.declare_dram_parameter("cos", [s, HD // 2], F32, isOutput=False)
    sinp = nc.declare_dram_parameter("sin", [s, HD // 2], F32, isOutput=False)
    wq = nc.declare_dram_parameter("wq", [D, DL], F32, isOutput=False)
    wk = nc.declare_dram_parameter("wk", [D, DL], F32, isOutput=False)
    wv = nc.declare_dram_parameter("wv", [D, DL], F32, isOutput=False)
    wo = nc.declare_dram_parameter("wo", [D, D], F32, isOutput=False)
    out = nc.declare_dram_parameter("out", [shard, D], F32, isOutput=True)
    if debug:
        dbg_q = nc.declare_dram_parameter("dbg_q", [b, DL, s], F32, isOutput=True)
        dbg_k = nc.declare_dram_parameter("dbg_k", [b, DL, s], F32, isOutput=True)
        dbg_v = nc.declare_dram_parameter("dbg_v", [b, DL, s], F32, isOutput=True)
        dbg_att = nc.declare_dram_parameter("dbg_att", [DL, b * s], F32, isOutput=True)
        dbg_sc = nc.declare_dram_parameter("dbg_sc", [HL, 128, chunk], F32, isOutput=True)
        dbg_pt = nc.declare_dram_parameter("dbg_pt", [HL, 128, chunk], F32, isOutput=True)
        dbg_pv = nc.declare_dram_parameter("dbg_pv", [HL, HD + 1, chunk], F32, isOutput=True)

    mperm = nc.inline_tensor(_perm_matrix().astype(np.float32), name="mperm")

    with tile.TileContext(nc) as tc:
        with (
            tc.tile_pool(name="dram", bufs=1, space="DRAM") as dram,
            tc.tile_pool(name="const", bufs=1) as cpool,
            tc.tile_pool(name="stage", bufs=1) as stpool,
            tc.tile_pool(name="xin", bufs=2) as xpool,
            tc.tile_pool(name="xbf", bufs=2) as xbpool,
            tc.tile_pool(name="xt", bufs=1) as xtpool,
            tc.tile_pool(name="qkv", bufs=2) as qkvpool,
            tc.tile_pool(name="rope", bufs=3) as ropepool,
            tc.tile_pool(name="pt", bufs=pt_bufs) as ptpool,
            tc.tile_pool(name="att", bufs=1) as attpool,
            tc.tile_pool(name="sig", bufs=4) as sigpool,
            tc.tile_pool(name="outp", bufs=2) as outpool,
            # PSUM is 8 banks total and pool slots are static:
            # psA holds tags "sc0"/"sc1" ([128, GMAX*chunk] = 3 banks each, 6
            # total; projection/rot/Wo accumulators borrow these tags), psB
            # holds 2 PV accumulators (1 bank each). 6 + 2 = 8.
            tc.tile_pool(name="psA", bufs=1, space="PSUM") as psA,
            tc.tile_pool(name="psB", bufs=2, space="PSUM") as psB,
        ):
            # ---- constants: weights (bf16), rotation matrix, cos/sin rows ----
            wq_sb = cpool.tile([128, dt8, DL], BF16, tag="wq")
            wk_sb = cpool.tile([128, dt8, DL], BF16, tag="wk")
            wv_sb = cpool.tile([128, dt8, DL], BF16, tag="wv")
            for wparam, wsb in ((wq, wq_sb), (wk, wk_sb), (wv, wv_sb)):
                wf = stpool.tile([128, dt8, DL], F32, tag="wstage")
                nc.sync.dma_start(wf[:], wparam.rearrange("(t p) m -> p t m", p=128))
                nc.scalar.copy(wsb[:], wf[:])

            wo_sb = cpool.tile([128, dt8, D], BF16, tag="wo")
            for dt in range(dt8):
                wof = stpool.tile([128, D], F32, tag="wostage")
                nc.sync.dma_start(wof[:], wo[dt * 128 : (dt + 1) * 128, :])
                nc.scalar.copy(wo_sb[:, dt, :], wof[:])

            mp_f = stpool.tile([DL, DL], F32, tag="mperm_f")
            nc.sync.dma_start(mp_f[:], mperm[:])
            mp_sb = cpool.tile([DL, DL], BF16, tag="mperm")
            nc.vector.tensor_copy(mp_sb[:], mp_f[:])

            # cos/sin: [s, 32] -> transposed, doubled rows -> [128, s] bf16
            st16 = s // 128
            cs128 = cpool.tile([128, s], BF16, tag="cs")
            sn128 = cpool.tile([128, s], BF16, tag="sn")
            for p, t128 in ((cosp, cs128), (sinp, sn128)):
                cf = stpool.tile([128, st16, HD // 2], F32, tag="cstage")
                nc.sync.dma_start(cf[:], p.rearrange("(t p) d -> p t d", p=128))
                cb = stpool.tile([128, st16, HD // 2], BF16, tag="cstage_b")
                nc.vector.tensor_copy(cb[:], cf[:])
                # XBAR transpose needs 128-divisible tiles: transpose 4
                # s-tiles (4*32 = 128 free) at once, then scatter row-blocks.
                for blk in range(st16 // 4):
                    ctmp = stpool.tile([128, 128], BF16, tag="cs_t")
                    nc.sync.dma_start_transpose(
                        out=ctmp[:], in_=cb[:, blk * 4 : (blk + 1) * 4, :]
                    )
                    for j in range(4):
                        st = blk * 4 + j
                        nc.sync.dma_start(
                            t128[0:32, st * 128 : (st + 1) * 128],
                            ctmp[j * 32 : (j + 1) * 32, :],
                        )
                for r in (32, 64, 96):
                    nc.sync.dma_start(t128[r : r + 32, :], t128[0:32, :])

            biasc = cpool.tile([128, 1], F32, tag="biasc")
            nc.vector.memset(biasc[:], EXP_BIAS)

            attnout = attpool.tile([DL, b * s], BF16, tag="attnout")

            for bi in range(b):
                # ---- x^T (bf16) via cast + DMA transpose ----
                xt_sb = xtpool.tile([128, dt8, s], BF16, tag="xt")
                for st in range(st16):
                    xf = xpool.tile([128, D], F32, tag="xf")
                    nc.sync.dma_start(xf[:], x[bi, st * 128 : (st + 1) * 128, :])
                    xb_ = xbpool.tile([128, D], BF16, tag="xb")
                    nc.vector.tensor_copy(xb_[:], xf[:])
                    for dt in range(dt8):
                        nc.sync.dma_start_transpose(
                            out=xt_sb[:, dt, st * 128 : (st + 1) * 128],
                            in_=xb_[:, dt * 128 : (dt + 1) * 128],
                        )

                # ---- q,k projections + RoPE; v projection + transpose ----
                q_rope = qkvpool.tile([DL, s], BF16, tag="q_rope")
                k_rope = qkvpool.tile([DL, s], BF16, tag="k_rope")
                vt_sb = qkvpool.tile([DL, s], BF16, tag="vt")
                for wsb, dst, is_v in (
                    (wq_sb, q_rope, False),
                    (wk_sb, k_rope, False),
                    (wv_sb, vt_sb, True),
                ):
                    for ch in range(nch):
                        cols = slice(ch * chunk, (ch + 1) * chunk)
                        ps = psA.tile([128, chunk], F32, tag="sc0")
                        for dt in range(dt8):
                            nc.tensor.matmul(
                                ps[:],
                                wsb[:, dt, :],
                                xt_sb[:, dt, cols],
                                start=(dt == 0),
                                stop=(dt == dt8 - 1),
                            )
                        if is_v:
                            nc.scalar.copy(dst[:, cols], ps[:])
                        else:
                            tsb = ropepool.tile([128, chunk], BF16, tag="tsb")
                            nc.scalar.copy(tsb[:], ps[:])
                            rps = psA.tile([128, chunk], F32, tag="sc1")
                            nc.tensor.matmul(
                                rps[:], mp_sb[:], tsb[:], start=True, stop=True
                            )
                            m1 = ropepool.tile([128, chunk], BF16, tag="m1")
                            nc.vector.tensor_tensor(
                                m1[:], ps[:], cs128[:, cols], mybir.AluOpType.mult
                            )
                            m2 = ropepool.tile([128, chunk], BF16, tag="m2")
                            nc.vector.tensor_tensor(
                                m2[:], rps[:], sn128[:, cols], mybir.AluOpType.mult
                            )
                            nc.vector.tensor_tensor(
                                dst[:, cols], m1[:], m2[:], mybir.AluOpType.add
                            )

                if debug:
                    for name, tl in (("dbg_q", q_rope), ("dbg_k", k_rope), ("dbg_v", vt_sb)):
                        df = outpool.tile([DL, s], F32, tag="dbgf")
                        nc.vector.tensor_copy(df[:], tl[:])
                        nc.sync.dma_start(
                            {"dbg_q": dbg_q, "dbg_k": dbg_k, "dbg_v": dbg_v}[name][bi],
                            df[:],
                        )

                # v_aug [s_k, hd+1] blocks (ones column fuses the softmax sum)
                v_sb = qkvpool.tile([128, kt, HL, HD + 1], BF16, tag="v_sb")
                nc.vector.memset(v_sb[:, :, :, HD : HD + 1], 1.0)
                # dma_start_transpose needs a contiguous destination: go
                # through a [128,128] tmp, then strided-copy per head.
                for ktt in range(kt):
                    vtmp = ropepool.tile([128, 128], BF16, tag="vtmp")
                    nc.sync.dma_start_transpose(
                        out=vtmp[:],
                        in_=vt_sb[:, ktt * 128 : (ktt + 1) * 128],
                    )
                    for h in range(HL):
                        nc.sync.dma_start(
                            v_sb[:, ktt, h, 0:HD],
                            vtmp[:, h * HD : (h + 1) * HD],
                        )

                # ---- attention: transposed scores -> exp -> PV (+sigma) ----
                for ch in range(nch):
                    cols = slice(ch * chunk, (ch + 1) * chunk)
                    pts = {}
                    for gi, (k0, glen) in enumerate(groups):
                        for h in range(HL):
                            rows = slice(h * HD, (h + 1) * HD)
                            sg = psA.tile([128, GMAX, chunk], F32, tag=f"sc{h}")
                            for j in range(glen):
                                ktt = k0 + j
                                nc.tensor.matmul(
                                    sg[:, j, :],
                                    k_rope[rows, ktt * 128 : (ktt + 1) * 128],
                                    q_rope[rows, cols],
                                    start=True,
                                    stop=True,
                                )
                            pt = ptpool.tile([128, GMAX, chunk], BF16, tag="pt")
                            nc.scalar.activation(
                                pt[:, :glen, :],
                                sg[:, :glen, :],
                                mybir.ActivationFunctionType.Exp,
                                bias=biasc[:],
                                scale=EXP_SCALE,
                            )
                            pts[(gi, h)] = pt
                            if debug and bi == 0 and ch == 0 and gi == 0:
                                dsc = outpool.tile([128, chunk], F32, tag="dbgsc")
                                nc.vector.tensor_copy(dsc[:], sg[:, 0, :])
                                nc.sync.dma_start(dbg_sc[h], dsc[:])
                                dpt = outpool.tile([128, chunk], F32, tag="dbgpt")
                                nc.vector.tensor_copy(dpt[:], pt[:, 0, :])
                                nc.sync.dma_start(dbg_pt[h], dpt[:])
                    for h in range(HL):
                        # accumulate per-group into short psum chains, then
                        # reduce the groups on DVE in SBUF
                        pv = sigpool.tile([HD + 1, chunk], F32, tag="pvacc")
                        for gi, (k0, glen) in enumerate(groups):
                            pt = pts[(gi, h)]
                            pvg = psB.tile([HD + 1, chunk], F32, tag="pv")
                            for j in range(glen):
                                nc.tensor.matmul(
                                    pvg[:],
                                    v_sb[:, k0 + j, h, :],
                                    pt[:, j, :],
                                    start=(j == 0),
                                    stop=(j == glen - 1),
                                )
                            if gi == 0:
                                nc.vector.tensor_copy(pv[:], pvg[:])
                            else:
                                nc.vector.tensor_tensor(
                                    pv[:], pv[:], pvg[:], mybir.AluOpType.add
                                )
                        if debug and bi == 0 and ch == 0:
                            dpv = outpool.tile([HD + 1, chunk], F32, tag="dbgpv")
                            nc.vector.tensor_copy(dpv[:], pv[:])
                            nc.sync.dma_start(dbg_pv[h], dpv[:])
                        # 1/sigma: reciprocal of the ones-column row, then
                        # log2-doubling DMA broadcast down the partitions.
                        bcast = sigpool.tile([HD + 1, chunk], F32, tag="bcast")
                        nc.vector.reciprocal(
                            bcast[HD : HD + 1, :], pv[HD : HD + 1, :]
                        )
                        nc.sync.dma_start(bcast[0:1, :], bcast[HD : HD + 1, :])
                        kk = 1
                        while kk < HD:
                            nc.sync.dma_start(bcast[kk : 2 * kk, :], bcast[0:kk, :])
                            kk *= 2
                        oh = sigpool.tile([HD, chunk], BF16, tag="oh")
                        nc.vector.tensor_tensor(
                            oh[:], pv[0:HD, :], bcast[0:HD, :], mybir.AluOpType.mult
                        )
                        nc.sync.dma_start(
                            attnout[
                                h * HD : (h + 1) * HD,
                                bi * s + ch * chunk : bi * s + (ch + 1) * chunk,
                            ],
                            oh[:],
                        )

            if debug:
                daf = outpool.tile([DL, b * s], F32, tag="dbga")
                nc.vector.tensor_copy(daf[:], attnout[:])
                nc.sync.dma_start(dbg_att[:], daf[:])

            # ---- AllToAll: attnout^T [128, b*s] -> per-shard [1024, shard] ----
            a2a_in = dram.tile([N_CORES, DL, shard], BF16, tag="a2a_in")
            a2a_out = dram.tile([N_CORES, DL, shard], BF16, tag="a2a_out")
            for j in range(N_CORES):
                nc.sync.dma_start(a2a_in[j], attnout[:, j * shard : (j + 1) * shard])
            nc.gpsimd.collective_compute(
                "AllToAll",
                mybir.AluOpType.bypass,
                replica_groups=[list(range(N_CORES))],
                ins=[a2a_in.opt()],
                outs=[a2a_out.opt()],
            )
            recv = cpool.tile([DL, N_CORES, shard], BF16, tag="recv")
            for i in range(N_CORES):
                nc.sync.dma_start(recv[:, i, :], a2a_out[i])

            # ---- output projection for this core's shard ----
            for j in range(shard // 128):
                for nco in range(D // 512):
                    wps = psA.tile([128, 512], F32, tag="sc0")
                    for i in range(N_CORES):
                        nc.tensor.matmul(
                            wps[:],
                            recv[:, i, j * 128 : (j + 1) * 128],
                            wo_sb[:, i, nco * 512 : (nco + 1) * 512],
                            start=(i == 0),
                            stop=(i == N_CORES - 1),
                        )
                    osb = outpool.tile([128, 512], F32, tag="osb")
                    nc.vector.tensor_copy(osb[:], wps[:])
                    nc.sync.dma_start(
                        out[j * 128 : (j + 1) * 128, nco * 512 : (nco + 1) * 512],
                        osb[:],
                    )

    split_excess_waits(nc)
    return nc


def make_in_maps(x, cos, sin, Wq, Wk, Wv, Wo, b, s):
    x = np.ascontiguousarray(x, dtype=np.float32)
    in_maps = []
    for c in range(N_CORES):
        cs = slice(c * DL, (c + 1) * DL)
        in_maps.append(
            {
                "x": x,
                "cos": np.ascontiguousarray(cos, dtype=np.float32),
                "sin": np.ascontiguousarray(sin, dtype=np.float32),
                "wq": np.ascontiguousarray(Wq[:, cs], dtype=np.float32),
                "wk": np.ascontiguousarray(Wk[:, cs], dtype=np.float32),
                "wv": np.ascontiguousarray(Wv[:, cs], dtype=np.float32),
                "wo": np.ascontiguousarray(Wo, dtype=np.float32),
            }
        )
    return in_maps


_NC_CACHE = {}


def run(x, cos, sin, Wq, Wk, Wv, Wo, trace=False, chunk=512, pt_bufs=12):
    b, s, _ = x.shape
    key = (b, s, chunk, pt_bufs)
    if key not in _NC_CACHE:
        _NC_CACHE[key] = build_nc(b=b, s=s, chunk=chunk, pt_bufs=pt_bufs)
    nc = _NC_CACHE[key]
    in_maps = make_in_maps(x, cos, sin, Wq, Wk, Wv, Wo, b, s)
    res = run_bass_kernel_spmd(nc, in_maps, list(range(N_CORES)), trace=trace)
    shard = b * s // N_CORES
    full = np.concatenate([res.results[c]["out"] for c in range(N_CORES)], axis=0)
    return full.reshape(b, s, D), res


def kernel(x, cos, sin, Wq, Wk, Wv, Wo):
    out, _ = run(
        np.asarray(x), np.asarray(cos), np.asarray(sin),
        np.asarray(Wq), np.asarray(Wk), np.asarray(Wv), np.asarray(Wo),
    )
    return out.astype(np.float32)


# revision 34
# speedup vs baseline: 1.5999x; 1.5999x over previous
"""Multi-head attention with RoPE on 8 Trainium2 NeuronCores.

Problem: x[2,2048,1024] -> MHA(16 heads, hd=64, NeoX RoPE, non-causal) -> out.

Sharding: tensor-parallel over heads. Each core owns 2 heads:
  - computes q^T,k^T (RoPE'd) and v for its heads over the full sequence
    (weights column-sliced on host; x replicated),
  - flash-style attention entirely on-chip with *transposed* scores
    [s_k, s_q] so the softmax denominator comes from a fused ones-column
    in V (no P transpose, no row-max pass; a constant bias inside the exp
    activation keeps the range safe),
  - one small AllToAll redistributes attention outputs so each core holds
    all 1024 attn dims for its 512-row output shard,
  - local Wo matmul produces the shard; host concatenates shards.

All matmuls run in bf16 (fp32 PSUM accumulation); rel-err tolerance is 2e-2.
"""

import sys

sys.path.insert(0, "/opt/trn_rl_repo")

import numpy as np  # noqa: E402

import concourse.bass as bass  # noqa: E402
import concourse.mybir as mybir  # noqa: E402
import concourse.tile as tile  # noqa: E402
from concourse.bass_utils import run_bass_kernel_spmd  # noqa: E402

N_CORES = 8
D = 1024
H = 16
HD = 64
HL = H // N_CORES  # local heads per core
DL = HL * HD  # 128 local attn dims
EXP_SCALE = 0.125  # 1/sqrt(hd)
EXP_BIAS = -24.0  # exp(s/8 - 24): cancels in softmax, keeps fp32 range safe
GMAX = 2  # scores-psum banks per (head, kt-group); 2*GMAX + 2 PV + 2 transpose <= 8

F32 = mybir.dt.float32
BF16 = mybir.dt.bfloat16


def _kt_groups(kt):
    """Split kt score-tiles into groups of <=GMAX (wider exp instructions)."""
    groups = []
    k0 = 0
    while k0 < kt:
        n3 = (kt - k0) // GMAX
        g = GMAX if n3 > 0 and (kt - k0) % GMAX != 1 else min(GMAX - 1, kt - k0)
        if (kt - k0) % GMAX == 0:
            g = GMAX
        groups.append((k0, g))
        k0 += g
    return groups


def _perm_matrix():
    """lhsT for the rotate_half matmul: qrot^T = lhsT.T @ q^T.

    Per head block at offset o: rot(q)[i] = -q[i+32] for i<32,
    rot(q)[i] = q[i-32] for 32<=i<64.
    """
    mt = np.zeros((DL, DL), dtype=np.float32)
    for o in (0, HD):
        for r in range(HD // 2):
            mt[o + r, o + r + HD // 2] = -1.0
            mt[o + r + HD // 2, o + r] = 1.0
    return np.ascontiguousarray(mt.T)


def split_excess_waits(nc, max_waits=1):
    """This container's walrus rejects >1 semaphore wait per instruction;
    split excess waits onto NoOp carriers on the same engine."""
    for bb in nc.m.functions[0].blocks:
        insts = bb.instructions
        idx = 0
        while idx < len(insts):
            ins = insts[idx]
            si = ins.sync_info
            if si is not None and si.on_wait and len(si.on_wait) > max_waits:
                ow = list(si.on_wait)
                si.on_wait = ow[-max_waits:]
                extra = ow[:-max_waits]
                k = 0
                while extra:
                    chunk, extra = extra[:max_waits], extra[max_waits:]
                    c = mybir.InstNoOp(name=f"{ins.name}-ws{k}", ins=[], outs=[])
                    c.engine = ins.engine
                    c.sync_info = mybir.SyncInfo(on_wait=chunk, on_update=[])
                    nc.register_instruction(c)
                    insts.insert(idx, c)
                    idx += 1
                    k += 1
            idx += 1


def build_nc(b=2, s=2048, chunk=512, pt_bufs=18, debug=False):
    kt = s // 128  # score tiles along s_k per batch
    nch = s // chunk  # s_q chunks per batch
    shard = b * s // N_CORES  # output rows per core
    groups = _kt_groups(kt)
    dt8 = D // 128  # contraction tiles for projections

    nc = bass.Bass()
    x = nc.declare_dram_parameter("x", [b, s, D], F32, isOutput=False)
    cosp = nc.declare_dram_parameter("cos", [s, HD // 2], F32, isOutput=False)
    sinp = nc.declare_dram_parameter("sin", [s, HD // 2], F32, isOutput=False)
    wq = nc.declare_dram_parameter("wq", [D, DL], F32, isOutput=False)
    wk = nc.declare_dram_parameter("wk", [D, DL], F32, isOutput=False)
    wv = nc.declare_dram_parameter("wv", [D, DL], F32, isOutput=False)
    wo = nc.declare_dram_parameter("wo", [D, D], F32, isOutput=False)
    out = nc.declare_dram_parameter("out", [shard, D], F32, isOutput=True)
    if debug:
        dbg_q = nc.declare_dram_parameter("dbg_q", [b, DL, s], F32, isOutput=True)
        dbg_k = nc.declare_dram_parameter("dbg_k", [b, DL, s], F32, isOutput=True)
        dbg_v = nc.declare_dram_parameter("dbg_v", [b, DL, s], F32, isOutput=True)
        dbg_att = nc.declare_dram_parameter("dbg_att", [DL, b * s], F32, isOutput=True)
        dbg_sc = nc.declare_dram_parameter("dbg_sc", [HL, 128, chunk], F32, isOutput=True)
        dbg_pt = nc.declare_dram_parameter("dbg_pt", [HL, 128, chunk], F32, isOutput=True)
        dbg_pv = nc.declare_dram_parameter("dbg_pv", [HL, HD + 1, chunk], F32, isOutput=True)

    mperm = nc.inline_tensor(_perm_matrix().astype(np.float32), name="mperm")
    ident = nc.inline_tensor(np.eye(128, dtype=np.float32), name="ident128")

    with tile.TileContext(nc) as tc:
        with (
            tc.tile_pool(name="dram", bufs=1, space="DRAM") as dram,
            tc.tile_pool(name="const", bufs=1) as cpool,
            tc.tile_pool(name="stage", bufs=1) as stpool,
            tc.tile_pool(name="xin", bufs=2) as xpool,
            tc.tile_pool(name="xbf", bufs=2) as xbpool,
            tc.tile_pool(name="xt", bufs=1) as xtpool,
            tc.tile_pool(name="qkv", bufs=2) as qkvpool,
            tc.tile_pool(name="rope", bufs=3) as ropepool,
            tc.tile_pool(name="pt", bufs=pt_bufs) as ptpool,
            tc.tile_pool(name="att", bufs=1) as attpool,
            tc.tile_pool(name="sig", bufs=4) as sigpool,
            tc.tile_pool(name="outp", bufs=2) as outpool,
            # PSUM is 8 banks total and pool slots are static:
            # psA holds tags "sc0"/"sc1" ([128, GMAX*chunk] = 3 banks each, 6
            # total; projection/rot/Wo accumulators borrow these tags), psB
            # holds 2 PV accumulators (1 bank each). 6 + 2 = 8.
            tc.tile_pool(name="psA", bufs=1, space="PSUM") as psA,
            tc.tile_pool(name="psB", bufs=2, space="PSUM") as psB,
            tc.tile_pool(name="psC", bufs=2, space="PSUM") as psC,
        ):
            # ---- constants: weights (bf16), rotation matrix, cos/sin rows ----
            wq_sb = cpool.tile([128, dt8, DL], BF16, tag="wq")
            wk_sb = cpool.tile([128, dt8, DL], BF16, tag="wk")
            wv_sb = cpool.tile([128, dt8, DL], BF16, tag="wv")
            for wparam, wsb in ((wq, wq_sb), (wk, wk_sb), (wv, wv_sb)):
                wf = stpool.tile([128, dt8, DL], F32, tag="wstage")
                nc.sync.dma_start(wf[:], wparam.rearrange("(t p) m -> p t m", p=128))
                nc.scalar.copy(wsb[:], wf[:])

            wo_sb = cpool.tile([128, dt8, D], BF16, tag="wo")
            for dt in range(dt8):
                wof = stpool.tile([128, D], F32, tag="wostage")
                nc.sync.dma_start(wof[:], wo[dt * 128 : (dt + 1) * 128, :])
                nc.scalar.copy(wo_sb[:, dt, :], wof[:])

            mp_f = stpool.tile([DL, DL], F32, tag="mperm_f")
            nc.sync.dma_start(mp_f[:], mperm[:])
            mp_sb = cpool.tile([DL, DL], BF16, tag="mperm")
            nc.vector.tensor_copy(mp_sb[:], mp_f[:])

            id_f = stpool.tile([128, 128], F32, tag="ident_f")
            nc.sync.dma_start(id_f[:], ident[:])
            id_sb = cpool.tile([128, 128], BF16, tag="ident")
            nc.vector.tensor_copy(id_sb[:], id_f[:])

            # cos/sin: [s, 32] -> transposed, doubled rows -> [128, s] bf16
            st16 = s // 128
            cs128 = cpool.tile([128, s], BF16, tag="cs")
            sn128 = cpool.tile([128, s], BF16, tag="sn")
            for p, t128 in ((cosp, cs128), (sinp, sn128)):
                cf = stpool.tile([128, st16, HD // 2], F32, tag="cstage")
                nc.sync.dma_start(cf[:], p.rearrange("(t p) d -> p t d", p=128))
                cb = stpool.tile([128, st16, HD // 2], BF16, tag="cstage_b")
                nc.vector.tensor_copy(cb[:], cf[:])
                # XBAR transpose needs 128-divisible tiles: transpose 4
                # s-tiles (4*32 = 128 free) at once, then scatter row-blocks.
                for blk in range(st16 // 4):
                    ctmp = stpool.tile([128, 128], BF16, tag="cs_t")
                    nc.sync.dma_start_transpose(
                        out=ctmp[:], in_=cb[:, blk * 4 : (blk + 1) * 4, :]
                    )
                    for j in range(4):
                        st = blk * 4 + j
                        nc.sync.dma_start(
                            t128[0:32, st * 128 : (st + 1) * 128],
                            ctmp[j * 32 : (j + 1) * 32, :],
                        )
                for r in (32, 64, 96):
                    nc.sync.dma_start(t128[r : r + 32, :], t128[0:32, :])

            biasc = cpool.tile([128, 1], F32, tag="biasc")
            nc.vector.memset(biasc[:], EXP_BIAS)

            attnout = attpool.tile([DL, b * s], BF16, tag="attnout")

            for bi in range(b):
                # ---- x^T (bf16) via cast + DMA transpose ----
                xt_sb = xtpool.tile([128, dt8, s], BF16, tag="xt")
                for st in range(st16):
                    xf = xpool.tile([128, D], F32, tag="xf")
                    nc.sync.dma_start(xf[:], x[bi, st * 128 : (st + 1) * 128, :])
                    xb_ = xbpool.tile([128, D], BF16, tag="xb")
                    nc.vector.tensor_copy(xb_[:], xf[:])
                    # transpose via identity matmul: out = x_tile.T @ I
                    for dt in range(dt8):
                        tps = psC.tile([128, 128], F32, tag="tp")
                        nc.tensor.matmul(
                            tps[:],
                            xb_[:, dt * 128 : (dt + 1) * 128],
                            id_sb[:],
                            start=True,
                            stop=True,
                        )
                        nc.vector.tensor_copy(
                            xt_sb[:, dt, st * 128 : (st + 1) * 128], tps[:]
                        )

                # ---- q,k projections + RoPE; v projection + transpose ----
                q_rope = qkvpool.tile([DL, s], BF16, tag="q_rope")
                k_rope = qkvpool.tile([DL, s], BF16, tag="k_rope")
                vt_sb = qkvpool.tile([DL, s], BF16, tag="vt")
                for wsb, dst, is_v in (
                    (wq_sb, q_rope, False),
                    (wk_sb, k_rope, False),
                    (wv_sb, vt_sb, True),
                ):
                    for ch in range(nch):
                        cols = slice(ch * chunk, (ch + 1) * chunk)
                        ps = psA.tile([128, chunk], F32, tag="sc0")
                        for dt in range(dt8):
                            nc.tensor.matmul(
                                ps[:],
                                wsb[:, dt, :],
                                xt_sb[:, dt, cols],
                                start=(dt == 0),
                                stop=(dt == dt8 - 1),
                            )
                        if is_v:
                            nc.scalar.copy(dst[:, cols], ps[:])
                        else:
                            tsb = ropepool.tile([128, chunk], BF16, tag="tsb")
                            nc.scalar.copy(tsb[:], ps[:])
                            rps = psA.tile([128, chunk], F32, tag="sc1")
                            nc.tensor.matmul(
                                rps[:], mp_sb[:], tsb[:], start=True, stop=True
                            )
                            m1 = ropepool.tile([128, chunk], BF16, tag="m1")
                            nc.vector.tensor_tensor(
                                m1[:], ps[:], cs128[:, cols], mybir.AluOpType.mult
                            )
                            m2 = ropepool.tile([128, chunk], BF16, tag="m2")
                            nc.vector.tensor_tensor(
                                m2[:], rps[:], sn128[:, cols], mybir.AluOpType.mult
                            )
                            nc.vector.tensor_tensor(
                                dst[:, cols], m1[:], m2[:], mybir.AluOpType.add
                            )

                if debug:
                    for name, tl in (("dbg_q", q_rope), ("dbg_k", k_rope), ("dbg_v", vt_sb)):
                        df = outpool.tile([DL, s], F32, tag="dbgf")
                        nc.vector.tensor_copy(df[:], tl[:])
                        nc.sync.dma_start(
                            {"dbg_q": dbg_q, "dbg_k": dbg_k, "dbg_v": dbg_v}[name][bi],
                            df[:],
                        )

                # v_aug [s_k, hd+1] blocks (ones column fuses the softmax sum)
                v_sb = qkvpool.tile([128, kt, HL, HD + 1], BF16, tag="v_sb")
                nc.vector.memset(v_sb[:, :, :, HD : HD + 1], 1.0)
                # transpose v^T -> v via identity matmul; the psum->sbuf
                # copies scatter the two head halves into the v_aug layout.
                for ktt in range(kt):
                    vps = psC.tile([128, 128], F32, tag="tp")
                    nc.tensor.matmul(
                        vps[:],
                        vt_sb[:, ktt * 128 : (ktt + 1) * 128],
                        id_sb[:],
                        start=True,
                        stop=True,
                    )
                    for h in range(HL):
                        nc.vector.tensor_copy(
                            v_sb[:, ktt, h, 0:HD], vps[:, h * HD : (h + 1) * HD]
                        )

                # ---- attention: transposed scores -> exp -> PV (+sigma) ----
                for ch in range(nch):
                    cols = slice(ch * chunk, (ch + 1) * chunk)
                    pts = {}
                    for gi, (k0, glen) in enumerate(groups):
                        for h in range(HL):
                            rows = slice(h * HD, (h + 1) * HD)
                            sg = psA.tile([128, GMAX, chunk], F32, tag=f"sc{h}")
                            for j in range(glen):
                                ktt = k0 + j
                                nc.tensor.matmul(
                                    sg[:, j, :],
                                    k_rope[rows, ktt * 128 : (ktt + 1) * 128],
                                    q_rope[rows, cols],
                                    start=True,
                                    stop=True,
                                )
                            pt = ptpool.tile([128, GMAX, chunk], BF16, tag="pt")
                            nc.scalar.activation(
                                pt[:, :glen, :],
                                sg[:, :glen, :],
                                mybir.ActivationFunctionType.Exp,
                                bias=biasc[:],
                                scale=EXP_SCALE,
                            )
                            pts[(gi, h)] = pt
                            if debug and bi == 0 and ch == 0 and gi == 0:
                                dsc = outpool.tile([128, chunk], F32, tag="dbgsc")
                                nc.vector.tensor_copy(dsc[:], sg[:, 0, :])
                                nc.sync.dma_start(dbg_sc[h], dsc[:])
                                dpt = outpool.tile([128, chunk], F32, tag="dbgpt")
                                nc.vector.tensor_copy(dpt[:], pt[:, 0, :])
                                nc.sync.dma_start(dbg_pt[h], dpt[:])
                    for h in range(HL):
                        pv = psB.tile([HD + 1, chunk], F32, tag="pv")
                        for gi, (k0, glen) in enumerate(groups):
                            pt = pts[(gi, h)]
                            for j in range(glen):
                                ktt = k0 + j
                                nc.tensor.matmul(
                                    pv[:],
                                    v_sb[:, ktt, h, :],
                                    pt[:, j, :],
                                    start=(ktt == 0),
                                    stop=(ktt == kt - 1),
                                )
                        if debug and bi == 0 and ch == 0:
                            dpv = outpool.tile([HD + 1, chunk], F32, tag="dbgpv")
                            nc.vector.tensor_copy(dpv[:], pv[:])
                            nc.sync.dma_start(dbg_pv[h], dpv[:])
                        # 1/sigma: fast reciprocal of the ones-column row, then
                        # log2-doubling DMA broadcast down the partitions
                        # (on the gpsimd queue to keep the sync queue free).
                        bcast = sigpool.tile([HD + 1, chunk], F32, tag="bcast")
                        nc.vector.reciprocal(
                            bcast[HD : HD + 1, :], pv[HD : HD + 1, :]
                        )
                        nc.gpsimd.dma_start(bcast[0:1, :], bcast[HD : HD + 1, :])
                        kk = 1
                        while kk < HD:
                            nc.gpsimd.dma_start(bcast[kk : 2 * kk, :], bcast[0:kk, :])
                            kk *= 2
                        oh = sigpool.tile([HD, chunk], BF16, tag="oh")
                        nc.vector.tensor_tensor(
                            oh[:], pv[0:HD, :], bcast[0:HD, :], mybir.AluOpType.mult
                        )
                        nc.gpsimd.dma_start(
                            attnout[
                                h * HD : (h + 1) * HD,
                                bi * s + ch * chunk : bi * s + (ch + 1) * chunk,
                            ],
                            oh[:],
                        )

            if debug:
                daf = outpool.tile([DL, b * s], F32, tag="dbga")
                nc.vector.tensor_copy(daf[:], attnout[:])
                nc.sync.dma_start(dbg_att[:], daf[:])

            # ---- AllToAll: attnout^T [128, b*s] -> per-shard [1024, shard] ----
            a2a_in = dram.tile([N_CORES, DL, shard], BF16, tag="a2a_in")
            a2a_out = dram.tile([N_CORES, DL, shard], BF16, tag="a2a_out")
            for j in range(N_CORES):
                nc.sync.dma_start(a2a_in[j], attnout[:, j * shard : (j + 1) * shard])
            nc.gpsimd.collective_compute(
                "AllToAll",
                mybir.AluOpType.bypass,
                replica_groups=[list(range(N_CORES))],
                ins=[a2a_in.opt()],
                outs=[a2a_out.opt()],
            )
            recv = cpool.tile([DL, N_CORES, shard], BF16, tag="recv")
            for i in range(N_CORES):
                nc.sync.dma_start(recv[:, i, :], a2a_out[i])

            # ---- output projection for this core's shard ----
            for j in range(shard // 128):
                for nco in range(D // 512):
                    wps = psA.tile([128, 512], F32, tag="sc0")
                    for i in range(N_CORES):
                        nc.tensor.matmul(
                            wps[:],
                            recv[:, i, j * 128 : (j + 1) * 128],
                            wo_sb[:, i, nco * 512 : (nco + 1) * 512],
                            start=(i == 0),
                            stop=(i == N_CORES - 1),
                        )
                    osb = outpool.tile([128, 512], F32, tag="osb")
                    nc.vector.tensor_copy(osb[:], wps[:])
                    nc.sync.dma_start(
                        out[j * 128 : (j + 1) * 128, nco * 512 : (nco + 1) * 512],
                        osb[:],
                    )

    split_excess_waits(nc)
    return nc


def make_in_maps(x, cos, sin, Wq, Wk, Wv, Wo, b, s):
    x = np.ascontiguousarray(x, dtype=np.float32)
    in_maps = []
    for c in range(N_CORES):
        cs = slice(c * DL, (c + 1) * DL)
        in_maps.append(
            {
                "x": x,
                "cos": np.ascontiguousarray(cos, dtype=np.float32),
                "sin": np.ascontiguousarray(sin, dtype=np.float32),
                "wq": np.ascontiguousarray(Wq[:, cs], dtype=np.float32),
                "wk": np.ascontiguousarray(Wk[:, cs], dtype=np.float32),
                "wv": np.ascontiguousarray(Wv[:, cs], dtype=np.float32),
                "wo": np.ascontiguousarray(Wo, dtype=np.float32),
            }
        )
    return in_maps


_NC_CACHE = {}


def run(x, cos, sin, Wq, Wk, Wv, Wo, trace=False, chunk=512, pt_bufs=18):
    b, s, _ = x.shape
    key = (b, s, chunk, pt_bufs)
    if key not in _NC_CACHE:
        _NC_CACHE[key] = build_nc(b=b, s=s, chunk=chunk, pt_bufs=pt_bufs)
    nc = _NC_CACHE[key]
    in_maps = make_in_maps(x, cos, sin, Wq, Wk, Wv, Wo, b, s)
    res = run_bass_kernel_spmd(nc, in_maps, list(range(N_CORES)), trace=trace)
    shard = b * s // N_CORES
    full = np.concatenate([res.results[c]["out"] for c in range(N_CORES)], axis=0)
    return full.reshape(b, s, D), res


def kernel(x, cos, sin, Wq, Wk, Wv, Wo):
    out, _ = run(
        np.asarray(x), np.asarray(cos), np.asarray(sin),
        np.asarray(Wq), np.asarray(Wk), np.asarray(Wv), np.asarray(Wo),
    )
    return out.astype(np.float32)


# revision 37
# speedup vs baseline: 1.6943x; 1.0591x over previous
"""Multi-head attention with RoPE on 8 Trainium2 NeuronCores.

Problem: x[2,2048,1024] -> MHA(16 heads, hd=64, NeoX RoPE, non-causal) -> out.

Sharding: tensor-parallel over heads. Each core owns 2 heads:
  - computes q^T,k^T (RoPE'd) and v for its heads over the full sequence
    (weights column-sliced on host; x replicated),
  - flash-style attention entirely on-chip with *transposed* scores
    [s_k, s_q] so the softmax denominator comes from a fused ones-column
    in V (no P transpose, no row-max pass; a constant bias inside the exp
    activation keeps the range safe),
  - one small AllToAll redistributes attention outputs so each core holds
    all 1024 attn dims for its 512-row output shard,
  - local Wo matmul produces the shard; host concatenates shards.

All matmuls run in bf16 (fp32 PSUM accumulation); rel-err tolerance is 2e-2.
"""

import sys

sys.path.insert(0, "/opt/trn_rl_repo")

import numpy as np  # noqa: E402

import concourse.bass as bass  # noqa: E402
import concourse.mybir as mybir  # noqa: E402
import concourse.tile as tile  # noqa: E402
from concourse.bass_utils import run_bass_kernel_spmd  # noqa: E402

N_CORES = 8
D = 1024
H = 16
HD = 64
HL = H // N_CORES  # local heads per core
DL = HL * HD  # 128 local attn dims
EXP_SCALE = 0.125  # 1/sqrt(hd)
EXP_BIAS = -24.0  # exp(s/8 - 24): cancels in softmax, keeps fp32 range safe
GMAX = 2  # scores-psum banks per (head, kt-group); 2*GMAX + 2 PV + 2 transpose <= 8

F32 = mybir.dt.float32
BF16 = mybir.dt.bfloat16


def _kt_groups(kt):
    """Split kt score-tiles into groups of <=GMAX (wider exp instructions)."""
    groups = []
    k0 = 0
    while k0 < kt:
        n3 = (kt - k0) // GMAX
        g = GMAX if n3 > 0 and (kt - k0) % GMAX != 1 else min(GMAX - 1, kt - k0)
        if (kt - k0) % GMAX == 0:
            g = GMAX
        groups.append((k0, g))
        k0 += g
    return groups


def _perm_matrix():
    """lhsT for the rotate_half matmul: qrot^T = lhsT.T @ q^T.

    Per head block at offset o: rot(q)[i] = -q[i+32] for i<32,
    rot(q)[i] = q[i-32] for 32<=i<64.
    """
    mt = np.zeros((DL, DL), dtype=np.float32)
    for o in (0, HD):
        for r in range(HD // 2):
            mt[o + r, o + r + HD // 2] = -1.0
            mt[o + r + HD // 2, o + r] = 1.0
    return np.ascontiguousarray(mt.T)


def split_excess_waits(nc, max_waits=1):
    """This container's walrus rejects >1 semaphore wait per instruction;
    split excess waits onto NoOp carriers on the same engine."""
    for bb in nc.m.functions[0].blocks:
        insts = bb.instructions
        idx = 0
        while idx < len(insts):
            ins = insts[idx]
            si = ins.sync_info
            if si is not None and si.on_wait and len(si.on_wait) > max_waits:
                ow = list(si.on_wait)
                si.on_wait = ow[-max_waits:]
                extra = ow[:-max_waits]
                k = 0
                while extra:
                    chunk, extra = extra[:max_waits], extra[max_waits:]
                    c = mybir.InstNoOp(name=f"{ins.name}-ws{k}", ins=[], outs=[])
                    c.engine = ins.engine
                    c.sync_info = mybir.SyncInfo(on_wait=chunk, on_update=[])
                    nc.register_instruction(c)
                    insts.insert(idx, c)
                    idx += 1
                    k += 1
            idx += 1


def build_nc(b=2, s=2048, chunk=512, pt_bufs=18, debug=False):
    kt = s // 128  # score tiles along s_k per batch
    nch = s // chunk  # s_q chunks per batch
    shard = b * s // N_CORES  # output rows per core
    groups = _kt_groups(kt)
    dt8 = D // 128  # contraction tiles for projections

    nc = bass.Bass()
    x = nc.declare_dram_parameter("x", [b, s, D], F32, isOutput=False)
    cosp = nc.declare_dram_parameter("cos", [s, HD // 2], F32, isOutput=False)
    sinp = nc.declare_dram_parameter("sin", [s, HD // 2], F32, isOutput=False)
    wq = nc.declare_dram_parameter("wq", [D, DL], F32, isOutput=False)
    wk = nc.declare_dram_parameter("wk", [D, DL], F32, isOutput=False)
    wv = nc.declare_dram_parameter("wv", [D, DL], F32, isOutput=False)
    wo = nc.declare_dram_parameter("wo", [D, D], F32, isOutput=False)
    out = nc.declare_dram_parameter("out", [shard, D], F32, isOutput=True)
    if debug:
        dbg_q = nc.declare_dram_parameter("dbg_q", [b, DL, s], F32, isOutput=True)
        dbg_k = nc.declare_dram_parameter("dbg_k", [b, DL, s], F32, isOutput=True)
        dbg_v = nc.declare_dram_parameter("dbg_v", [b, DL, s], F32, isOutput=True)
        dbg_att = nc.declare_dram_parameter("dbg_att", [DL, b * s], F32, isOutput=True)
        dbg_sc = nc.declare_dram_parameter("dbg_sc", [HL, 128, chunk], F32, isOutput=True)
        dbg_pt = nc.declare_dram_parameter("dbg_pt", [HL, 128, chunk], F32, isOutput=True)
        dbg_pv = nc.declare_dram_parameter("dbg_pv", [HL, HD + 1, chunk], F32, isOutput=True)

    mperm = nc.inline_tensor(_perm_matrix().astype(np.float32), name="mperm")
    ident = nc.inline_tensor(np.eye(128, dtype=np.float32), name="ident128")

    with tile.TileContext(nc) as tc:
        with (
            tc.tile_pool(name="dram", bufs=1, space="DRAM") as dram,
            tc.tile_pool(name="const", bufs=1) as cpool,
            tc.tile_pool(name="stage", bufs=1) as stpool,
            tc.tile_pool(name="xin", bufs=3) as xpool,
            tc.tile_pool(name="xbf", bufs=3) as xbpool,
            tc.tile_pool(name="xt", bufs=1) as xtpool,
            tc.tile_pool(name="qkv", bufs=2) as qkvpool,
            tc.tile_pool(name="rope", bufs=3) as ropepool,
            tc.tile_pool(name="pt", bufs=pt_bufs) as ptpool,
            tc.tile_pool(name="att", bufs=1) as attpool,
            tc.tile_pool(name="sig", bufs=4) as sigpool,
            tc.tile_pool(name="outp", bufs=2) as outpool,
            # PSUM is 8 banks total and pool slots are static:
            # psA holds tags "sc0"/"sc1" ([128, GMAX*chunk] = 3 banks each, 6
            # total; projection/rot/Wo accumulators borrow these tags), psB
            # holds 2 PV accumulators (1 bank each). 6 + 2 = 8.
            tc.tile_pool(name="psA", bufs=1, space="PSUM") as psA,
            tc.tile_pool(name="psB", bufs=2, space="PSUM") as psB,
            tc.tile_pool(name="psC", bufs=2, space="PSUM") as psC,
        ):
            # ---- constants: weights (bf16), rotation matrix, cos/sin rows ----
            wq_sb = cpool.tile([128, dt8, DL], BF16, tag="wq")
            wk_sb = cpool.tile([128, dt8, DL], BF16, tag="wk")
            wv_sb = cpool.tile([128, dt8, DL], BF16, tag="wv")
            for wparam, wsb in ((wq, wq_sb), (wk, wk_sb), (wv, wv_sb)):
                wf = stpool.tile([128, dt8, DL], F32, tag="wstage")
                nc.sync.dma_start(wf[:], wparam.rearrange("(t p) m -> p t m", p=128))
                nc.scalar.copy(wsb[:], wf[:])

            wo_sb = cpool.tile([128, dt8, D], BF16, tag="wo")
            for dt in range(dt8):
                wof = stpool.tile([128, D], F32, tag="wostage")
                nc.sync.dma_start(wof[:], wo[dt * 128 : (dt + 1) * 128, :])
                nc.scalar.copy(wo_sb[:, dt, :], wof[:])

            mp_f = stpool.tile([DL, DL], F32, tag="mperm_f")
            nc.sync.dma_start(mp_f[:], mperm[:])
            mp_sb = cpool.tile([DL, DL], BF16, tag="mperm")
            nc.vector.tensor_copy(mp_sb[:], mp_f[:])

            id_f = stpool.tile([128, 128], F32, tag="ident_f")
            nc.sync.dma_start(id_f[:], ident[:])
            id_sb = cpool.tile([128, 128], BF16, tag="ident")
            nc.vector.tensor_copy(id_sb[:], id_f[:])

            # cos/sin: [s, 32] -> transposed, doubled rows -> [128, s] bf16
            st16 = s // 128
            cs128 = cpool.tile([128, s], BF16, tag="cs")
            sn128 = cpool.tile([128, s], BF16, tag="sn")
            for p, t128 in ((cosp, cs128), (sinp, sn128)):
                cf = stpool.tile([128, st16, HD // 2], F32, tag="cstage")
                nc.sync.dma_start(cf[:], p.rearrange("(t p) d -> p t d", p=128))
                cb = stpool.tile([128, st16, HD // 2], BF16, tag="cstage_b")
                nc.vector.tensor_copy(cb[:], cf[:])
                # XBAR transpose needs 128-divisible tiles: transpose 4
                # s-tiles (4*32 = 128 free) at once, then scatter row-blocks.
                for blk in range(st16 // 4):
                    ctmp = stpool.tile([128, 128], BF16, tag="cs_t")
                    nc.sync.dma_start_transpose(
                        out=ctmp[:], in_=cb[:, blk * 4 : (blk + 1) * 4, :]
                    )
                    for j in range(4):
                        st = blk * 4 + j
                        nc.sync.dma_start(
                            t128[0:32, st * 128 : (st + 1) * 128],
                            ctmp[j * 32 : (j + 1) * 32, :],
                        )
                for r in (32, 64, 96):
                    nc.sync.dma_start(t128[r : r + 32, :], t128[0:32, :])

            biasc = cpool.tile([128, 1], F32, tag="biasc")
            nc.vector.memset(biasc[:], EXP_BIAS)

            attnout = attpool.tile([DL, b * s], BF16, tag="attnout")

            for bi in range(b):
                # ---- x^T (bf16) via cast + DMA transpose ----
                xt_sb = xtpool.tile([128, dt8, s], BF16, tag="xt")
                for st in range(st16):
                    xf = xpool.tile([128, D], F32, tag="xf")
                    nc.sync.dma_start(xf[:], x[bi, st * 128 : (st + 1) * 128, :])
                    xb_ = xbpool.tile([128, D], BF16, tag="xb")
                    nc.vector.tensor_copy(xb_[:], xf[:])
                    # transpose via identity matmul: out = x_tile.T @ I.
                    # 4 transposes share one psum bank -> 1 batched copy.
                    for dt4 in range(dt8 // 4):
                        tps = psC.tile([128, 4, 128], F32, tag="tp")
                        for j in range(4):
                            dt = dt4 * 4 + j
                            nc.tensor.matmul(
                                tps[:, j, :],
                                xb_[:, dt * 128 : (dt + 1) * 128],
                                id_sb[:],
                                start=True,
                                stop=True,
                            )
                        nc.vector.tensor_copy(
                            xt_sb[:, dt4 * 4 : (dt4 + 1) * 4, st * 128 : (st + 1) * 128],
                            tps[:],
                        )

                # ---- q,k projections + RoPE; v projection + transpose ----
                q_rope = qkvpool.tile([DL, s], BF16, tag="q_rope")
                k_rope = qkvpool.tile([DL, s], BF16, tag="k_rope")
                vt_sb = qkvpool.tile([DL, s], BF16, tag="vt")
                for wsb, dst, is_v in (
                    (wq_sb, q_rope, False),
                    (wk_sb, k_rope, False),
                    (wv_sb, vt_sb, True),
                ):
                    for ch in range(nch):
                        cols = slice(ch * chunk, (ch + 1) * chunk)
                        ps = psA.tile([128, chunk], F32, tag="sc0")
                        for dt in range(dt8):
                            nc.tensor.matmul(
                                ps[:],
                                wsb[:, dt, :],
                                xt_sb[:, dt, cols],
                                start=(dt == 0),
                                stop=(dt == dt8 - 1),
                            )
                        if is_v:
                            nc.scalar.copy(dst[:, cols], ps[:])
                        else:
                            tsb = ropepool.tile([128, chunk], BF16, tag="tsb")
                            nc.scalar.copy(tsb[:], ps[:])
                            rps = psA.tile([128, chunk], F32, tag="sc1")
                            nc.tensor.matmul(
                                rps[:], mp_sb[:], tsb[:], start=True, stop=True
                            )
                            m1 = ropepool.tile([128, chunk], BF16, tag="m1")
                            nc.vector.tensor_tensor(
                                m1[:], ps[:], cs128[:, cols], mybir.AluOpType.mult
                            )
                            m2 = ropepool.tile([128, chunk], BF16, tag="m2")
                            nc.vector.tensor_tensor(
                                m2[:], rps[:], sn128[:, cols], mybir.AluOpType.mult
                            )
                            nc.vector.tensor_tensor(
                                dst[:, cols], m1[:], m2[:], mybir.AluOpType.add
                            )

                if debug:
                    for name, tl in (("dbg_q", q_rope), ("dbg_k", k_rope), ("dbg_v", vt_sb)):
                        df = outpool.tile([DL, s], F32, tag="dbgf")
                        nc.vector.tensor_copy(df[:], tl[:])
                        nc.sync.dma_start(
                            {"dbg_q": dbg_q, "dbg_k": dbg_k, "dbg_v": dbg_v}[name][bi],
                            df[:],
                        )

                # v_aug [s_k, hd+1] blocks (ones column fuses the softmax sum)
                v_sb = qkvpool.tile([128, kt, HL, HD + 1], BF16, tag="v_sb")
                nc.vector.memset(v_sb[:, :, :, HD : HD + 1], 1.0)
                # transpose v^T -> v via identity matmul; the psum->sbuf
                # copies scatter the two head halves into the v_aug layout.
                for kt4 in range(kt // 4):
                    vps = psC.tile([128, 4, 128], F32, tag="tp")
                    for j in range(4):
                        ktt = kt4 * 4 + j
                        nc.tensor.matmul(
                            vps[:, j, :],
                            vt_sb[:, ktt * 128 : (ktt + 1) * 128],
                            id_sb[:],
                            start=True,
                            stop=True,
                        )
                    for h in range(HL):
                        nc.vector.tensor_copy(
                            v_sb[:, kt4 * 4 : (kt4 + 1) * 4, h, 0:HD],
                            vps[:, :, h * HD : (h + 1) * HD],
                        )

                # ---- attention: transposed scores -> exp -> PV (+sigma) ----
                for ch in range(nch):
                    cols = slice(ch * chunk, (ch + 1) * chunk)
                    pts = {}
                    for gi, (k0, glen) in enumerate(groups):
                        for h in range(HL):
                            rows = slice(h * HD, (h + 1) * HD)
                            sg = psA.tile([128, GMAX, chunk], F32, tag=f"sc{h}")
                            for j in range(glen):
                                ktt = k0 + j
                                nc.tensor.matmul(
                                    sg[:, j, :],
                                    k_rope[rows, ktt * 128 : (ktt + 1) * 128],
                                    q_rope[rows, cols],
                                    start=True,
                                    stop=True,
                                )
                            pt = ptpool.tile([128, GMAX, chunk], BF16, tag="pt")
                            nc.scalar.activation(
                                pt[:, :glen, :],
                                sg[:, :glen, :],
                                mybir.ActivationFunctionType.Exp,
                                bias=biasc[:],
                                scale=EXP_SCALE,
                            )
                            pts[(gi, h)] = pt
                            if debug and bi == 0 and ch == 0 and gi == 0:
                                dsc = outpool.tile([128, chunk], F32, tag="dbgsc")
                                nc.vector.tensor_copy(dsc[:], sg[:, 0, :])
                                nc.sync.dma_start(dbg_sc[h], dsc[:])
                                dpt = outpool.tile([128, chunk], F32, tag="dbgpt")
                                nc.vector.tensor_copy(dpt[:], pt[:, 0, :])
                                nc.sync.dma_start(dbg_pt[h], dpt[:])
                    for h in range(HL):
                        pv = psB.tile([HD + 1, chunk], F32, tag="pv")
                        for gi, (k0, glen) in enumerate(groups):
                            pt = pts[(gi, h)]
                            for j in range(glen):
                                ktt = k0 + j
                                nc.tensor.matmul(
                                    pv[:],
                                    v_sb[:, ktt, h, :],
                                    pt[:, j, :],
                                    start=(ktt == 0),
                                    stop=(ktt == kt - 1),
                                )
                        if debug and bi == 0 and ch == 0:
                            dpv = outpool.tile([HD + 1, chunk], F32, tag="dbgpv")
                            nc.vector.tensor_copy(dpv[:], pv[:])
                            nc.sync.dma_start(dbg_pv[h], dpv[:])
                        # 1/sigma: fast reciprocal of the ones-column row, then
                        # log2-doubling DMA broadcast down the partitions
                        # (on the gpsimd queue to keep the sync queue free).
                        bcast = sigpool.tile([HD + 1, chunk], F32, tag="bcast")
                        nc.vector.reciprocal(
                            bcast[HD : HD + 1, :], pv[HD : HD + 1, :]
                        )
                        nc.gpsimd.dma_start(bcast[0:1, :], bcast[HD : HD + 1, :])
                        kk = 1
                        while kk < HD:
                            nc.gpsimd.dma_start(bcast[kk : 2 * kk, :], bcast[0:kk, :])
                            kk *= 2
                        oh = sigpool.tile([HD, chunk], BF16, tag="oh")
                        nc.vector.tensor_tensor(
                            oh[:], pv[0:HD, :], bcast[0:HD, :], mybir.AluOpType.mult
                        )
                        nc.gpsimd.dma_start(
                            attnout[
                                h * HD : (h + 1) * HD,
                                bi * s + ch * chunk : bi * s + (ch + 1) * chunk,
                            ],
                            oh[:],
                        )

            if debug:
                daf = outpool.tile([DL, b * s], F32, tag="dbga")
                nc.vector.tensor_copy(daf[:], attnout[:])
                nc.sync.dma_start(dbg_att[:], daf[:])

            # ---- AllToAll: attnout^T [128, b*s] -> per-shard [1024, shard] ----
            a2a_in = dram.tile([N_CORES, DL, shard], BF16, tag="a2a_in")
            a2a_out = dram.tile([N_CORES, DL, shard], BF16, tag="a2a_out")
            for j in range(N_CORES):
                nc.sync.dma_start(a2a_in[j], attnout[:, j * shard : (j + 1) * shard])
            nc.gpsimd.collective_compute(
                "AllToAll",
                mybir.AluOpType.bypass,
                replica_groups=[list(range(N_CORES))],
                ins=[a2a_in.opt()],
                outs=[a2a_out.opt()],
            )
            recv = cpool.tile([DL, N_CORES, shard], BF16, tag="recv")
            for i in range(N_CORES):
                nc.sync.dma_start(recv[:, i, :], a2a_out[i])

            # ---- output projection for this core's shard ----
            for j in range(shard // 128):
                for nco in range(D // 512):
                    wps = psA.tile([128, 512], F32, tag="sc0")
                    for i in range(N_CORES):
                        nc.tensor.matmul(
                            wps[:],
                            recv[:, i, j * 128 : (j + 1) * 128],
                            wo_sb[:, i, nco * 512 : (nco + 1) * 512],
                            start=(i == 0),
                            stop=(i == N_CORES - 1),
                        )
                    osb = outpool.tile([128, 512], F32, tag="osb")
                    nc.vector.tensor_copy(osb[:], wps[:])
                    nc.sync.dma_start(
                        out[j * 128 : (j + 1) * 128, nco * 512 : (nco + 1) * 512],
                        osb[:],
                    )

    split_excess_waits(nc)
    return nc


def make_in_maps(x, cos, sin, Wq, Wk, Wv, Wo, b, s):
    x = np.ascontiguousarray(x, dtype=np.float32)
    in_maps = []
    for c in range(N_CORES):
        cs = slice(c * DL, (c + 1) * DL)
        in_maps.append(
            {
                "x": x,
                "cos": np.ascontiguousarray(cos, dtype=np.float32),
                "sin": np.ascontiguousarray(sin, dtype=np.float32),
                "wq": np.ascontiguousarray(Wq[:, cs], dtype=np.float32),
                "wk": np.ascontiguousarray(Wk[:, cs], dtype=np.float32),
                "wv": np.ascontiguousarray(Wv[:, cs], dtype=np.float32),
                "wo": np.ascontiguousarray(Wo, dtype=np.float32),
            }
        )
    return in_maps


_NC_CACHE = {}


def run(x, cos, sin, Wq, Wk, Wv, Wo, trace=False, chunk=512, pt_bufs=18):
    b, s, _ = x.shape
    key = (b, s, chunk, pt_bufs)
    if key not in _NC_CACHE:
        _NC_CACHE[key] = build_nc(b=b, s=s, chunk=chunk, pt_bufs=pt_bufs)
    nc = _NC_CACHE[key]
    in_maps = make_in_maps(x, cos, sin, Wq, Wk, Wv, Wo, b, s)
    res = run_bass_kernel_spmd(nc, in_maps, list(range(N_CORES)), trace=trace)
    shard = b * s // N_CORES
    full = np.concatenate([res.results[c]["out"] for c in range(N_CORES)], axis=0)
    return full.reshape(b, s, D), res


def kernel(x, cos, sin, Wq, Wk, Wv, Wo):
    out, _ = run(
        np.asarray(x), np.asarray(cos), np.asarray(sin),
        np.asarray(Wq), np.asarray(Wk), np.asarray(Wv), np.asarray(Wo),
    )
    return out.astype(np.float32)


# revision 41
# speedup vs baseline: 1.7671x; 1.0429x over previous
"""Multi-head attention with RoPE on 8 Trainium2 NeuronCores.

Problem: x[2,2048,1024] -> MHA(16 heads, hd=64, NeoX RoPE, non-causal) -> out.

Sharding: tensor-parallel over heads. Each core owns 2 heads:
  - computes q^T,k^T (RoPE'd) and v for its heads over the full sequence
    (weights column-sliced on host; x replicated),
  - flash-style attention entirely on-chip with *transposed* scores
    [s_k, s_q] so the softmax denominator comes from a fused ones-column
    in V (no P transpose, no row-max pass; a constant bias inside the exp
    activation keeps the range safe),
  - one small AllToAll redistributes attention outputs so each core holds
    all 1024 attn dims for its 512-row output shard,
  - local Wo matmul produces the shard; host concatenates shards.

All matmuls run in bf16 (fp32 PSUM accumulation); rel-err tolerance is 2e-2.
"""

import sys

sys.path.insert(0, "/opt/trn_rl_repo")

import numpy as np  # noqa: E402

import concourse.bass as bass  # noqa: E402
import concourse.mybir as mybir  # noqa: E402
import concourse.tile as tile  # noqa: E402
from concourse.bass_utils import run_bass_kernel_spmd  # noqa: E402

N_CORES = 8
D = 1024
H = 16
HD = 64
HL = H // N_CORES  # local heads per core
DL = HL * HD  # 128 local attn dims
EXP_SCALE = 0.125  # 1/sqrt(hd)
EXP_BIAS = -24.0  # exp(s/8 - 24): cancels in softmax, keeps fp32 range safe
GMAX = 2  # scores-psum banks per (head, kt-group); 2*GMAX + 2 PV + 2 transpose <= 8

F32 = mybir.dt.float32
BF16 = mybir.dt.bfloat16


def _kt_groups(kt):
    """Split kt score-tiles into groups of <=GMAX (wider exp instructions)."""
    groups = []
    k0 = 0
    while k0 < kt:
        n3 = (kt - k0) // GMAX
        g = GMAX if n3 > 0 and (kt - k0) % GMAX != 1 else min(GMAX - 1, kt - k0)
        if (kt - k0) % GMAX == 0:
            g = GMAX
        groups.append((k0, g))
        k0 += g
    return groups


def _perm_matrix():
    """lhsT for the rotate_half matmul: qrot^T = lhsT.T @ q^T.

    Per head block at offset o: rot(q)[i] = -q[i+32] for i<32,
    rot(q)[i] = q[i-32] for 32<=i<64.
    """
    mt = np.zeros((DL, DL), dtype=np.float32)
    for o in (0, HD):
        for r in range(HD // 2):
            mt[o + r, o + r + HD // 2] = -1.0
            mt[o + r + HD // 2, o + r] = 1.0
    return np.ascontiguousarray(mt.T)


def split_excess_waits(nc, max_waits=1):
    """This container's walrus rejects >1 semaphore wait per instruction;
    split excess waits onto NoOp carriers on the same engine."""
    for bb in nc.m.functions[0].blocks:
        insts = bb.instructions
        idx = 0
        while idx < len(insts):
            ins = insts[idx]
            si = ins.sync_info
            if si is not None and si.on_wait and len(si.on_wait) > max_waits:
                ow = list(si.on_wait)
                si.on_wait = ow[-max_waits:]
                extra = ow[:-max_waits]
                k = 0
                while extra:
                    chunk, extra = extra[:max_waits], extra[max_waits:]
                    c = mybir.InstNoOp(name=f"{ins.name}-ws{k}", ins=[], outs=[])
                    c.engine = ins.engine
                    c.sync_info = mybir.SyncInfo(on_wait=chunk, on_update=[])
                    nc.register_instruction(c)
                    insts.insert(idx, c)
                    idx += 1
                    k += 1
            idx += 1


def build_nc(b=2, s=2048, chunk=512, pt_bufs=18, debug=False):
    kt = s // 128  # score tiles along s_k per batch
    nch = s // chunk  # s_q chunks per batch
    shard = b * s // N_CORES  # output rows per core
    groups = _kt_groups(kt)
    dt8 = D // 128  # contraction tiles for projections

    nc = bass.Bass()
    x = nc.declare_dram_parameter("x", [b, s, D], F32, isOutput=False)
    cosp = nc.declare_dram_parameter("cos", [s, HD // 2], F32, isOutput=False)
    sinp = nc.declare_dram_parameter("sin", [s, HD // 2], F32, isOutput=False)
    wq = nc.declare_dram_parameter("wq", [D, DL], F32, isOutput=False)
    wk = nc.declare_dram_parameter("wk", [D, DL], F32, isOutput=False)
    wv = nc.declare_dram_parameter("wv", [D, DL], F32, isOutput=False)
    wo = nc.declare_dram_parameter("wo", [D, D], F32, isOutput=False)
    out = nc.declare_dram_parameter("out", [shard, D], F32, isOutput=True)
    if debug:
        dbg_q = nc.declare_dram_parameter("dbg_q", [b, DL, s], F32, isOutput=True)
        dbg_k = nc.declare_dram_parameter("dbg_k", [b, DL, s], F32, isOutput=True)
        dbg_v = nc.declare_dram_parameter("dbg_v", [b, DL, s], F32, isOutput=True)
        dbg_att = nc.declare_dram_parameter("dbg_att", [DL, b * s], F32, isOutput=True)
        dbg_sc = nc.declare_dram_parameter("dbg_sc", [HL, 128, chunk], F32, isOutput=True)
        dbg_pt = nc.declare_dram_parameter("dbg_pt", [HL, 128, chunk], F32, isOutput=True)
        dbg_pv = nc.declare_dram_parameter("dbg_pv", [HL, HD + 1, chunk], F32, isOutput=True)

    mperm = nc.inline_tensor(_perm_matrix().astype(np.float32), name="mperm")
    ident = nc.inline_tensor(np.eye(128, dtype=np.float32), name="ident128")

    with tile.TileContext(nc) as tc:
        with (
            tc.tile_pool(name="dram", bufs=1, space="DRAM") as dram,
            tc.tile_pool(name="const", bufs=1) as cpool,
            tc.tile_pool(name="stage", bufs=1) as stpool,
            tc.tile_pool(name="xin", bufs=3) as xpool,
            tc.tile_pool(name="xbf", bufs=3) as xbpool,
            tc.tile_pool(name="xt", bufs=1) as xtpool,
            tc.tile_pool(name="qkv", bufs=2) as qkvpool,
            tc.tile_pool(name="rope", bufs=3) as ropepool,
            tc.tile_pool(name="pt", bufs=pt_bufs) as ptpool,
            tc.tile_pool(name="att", bufs=1) as attpool,
            tc.tile_pool(name="sig", bufs=4) as sigpool,
            tc.tile_pool(name="outp", bufs=2) as outpool,
            # PSUM is 8 banks total and pool slots are static:
            # psA holds tags "sc0"/"sc1" ([128, GMAX*chunk] = 3 banks each, 6
            # total; projection/rot/Wo accumulators borrow these tags), psB
            # holds 2 PV accumulators (1 bank each). 6 + 2 = 8.
            tc.tile_pool(name="psA", bufs=1, space="PSUM") as psA,
            tc.tile_pool(name="psB", bufs=2, space="PSUM") as psB,
            tc.tile_pool(name="psC", bufs=2, space="PSUM") as psC,
        ):
            # ---- constants: weights (bf16), rotation matrix, cos/sin rows ----
            wq_sb = cpool.tile([128, dt8, DL], BF16, tag="wq")
            wk_sb = cpool.tile([128, dt8, DL], BF16, tag="wk")
            wv_sb = cpool.tile([128, dt8, DL], BF16, tag="wv")
            for wparam, wsb in ((wq, wq_sb), (wk, wk_sb), (wv, wv_sb)):
                wf = stpool.tile([128, dt8, DL], F32, tag="wstage")
                nc.sync.dma_start(wf[:], wparam.rearrange("(t p) m -> p t m", p=128))
                nc.scalar.copy(wsb[:], wf[:])

            mp_f = stpool.tile([DL, DL], F32, tag="mperm_f")
            nc.sync.dma_start(mp_f[:], mperm[:])
            mp_sb = cpool.tile([DL, DL], BF16, tag="mperm")
            nc.vector.tensor_copy(mp_sb[:], mp_f[:])

            id_f = stpool.tile([128, 128], F32, tag="ident_f")
            nc.sync.dma_start(id_f[:], ident[:])
            id_sb = cpool.tile([128, 128], BF16, tag="ident")
            nc.vector.tensor_copy(id_sb[:], id_f[:])

            # cos/sin: [s, 32] -> transposed, doubled rows -> [128, s] bf16
            st16 = s // 128
            cs128 = cpool.tile([128, s], BF16, tag="cs")
            sn128 = cpool.tile([128, s], BF16, tag="sn")
            for p, t128 in ((cosp, cs128), (sinp, sn128)):
                cf = stpool.tile([128, st16, HD // 2], F32, tag="cstage")
                nc.sync.dma_start(cf[:], p.rearrange("(t p) d -> p t d", p=128))
                cb = stpool.tile([128, st16, HD // 2], BF16, tag="cstage_b")
                nc.vector.tensor_copy(cb[:], cf[:])
                # XBAR transpose needs 128-divisible tiles: transpose 4
                # s-tiles (4*32 = 128 free) at once, then scatter row-blocks.
                for blk in range(st16 // 4):
                    ctmp = stpool.tile([128, 128], BF16, tag="cs_t")
                    nc.sync.dma_start_transpose(
                        out=ctmp[:], in_=cb[:, blk * 4 : (blk + 1) * 4, :]
                    )
                    for j in range(4):
                        st = blk * 4 + j
                        nc.sync.dma_start(
                            t128[0:32, st * 128 : (st + 1) * 128],
                            ctmp[j * 32 : (j + 1) * 32, :],
                        )
                for r in (32, 64, 96):
                    nc.sync.dma_start(t128[r : r + 32, :], t128[0:32, :])

            biasc = cpool.tile([128, 1], F32, tag="biasc")
            nc.vector.memset(biasc[:], EXP_BIAS)

            attnout = attpool.tile([DL, b * s], BF16, tag="attnout")

            # Wo staging is off the startup critical path: emit after batch 0's
            # x pipeline in program order (scheduler fills DMA gaps with it).
            wo_sb = cpool.tile([128, dt8, D], BF16, tag="wo")

            for bi in range(b):
                # ---- x^T (bf16) via cast + DMA transpose ----
                xt_sb = xtpool.tile([128, dt8, s], BF16, tag="xt")
                for st in range(st16):
                    xf = xpool.tile([128, D], F32, tag="xf")
                    nc.sync.dma_start(xf[:], x[bi, st * 128 : (st + 1) * 128, :])
                    xb_ = xbpool.tile([128, D], BF16, tag="xb")
                    nc.vector.tensor_copy(xb_[:], xf[:])
                    # transpose via identity matmul: out = x_tile.T @ I.
                    # 4 transposes share one psum bank -> 1 batched copy.
                    for dt4 in range(dt8 // 4):
                        tps = psC.tile([128, 4, 128], F32, tag="tp")
                        for j in range(4):
                            dt = dt4 * 4 + j
                            nc.tensor.matmul(
                                tps[:, j, :],
                                xb_[:, dt * 128 : (dt + 1) * 128],
                                id_sb[:],
                                start=True,
                                stop=True,
                            )
                        nc.vector.tensor_copy(
                            xt_sb[:, dt4 * 4 : (dt4 + 1) * 4, st * 128 : (st + 1) * 128],
                            tps[:],
                        )

                # ---- q,k projections + RoPE; v projection + transpose ----
                q_rope = qkvpool.tile([DL, s], BF16, tag="q_rope")
                k_rope = qkvpool.tile([DL, s], BF16, tag="k_rope")
                vt_sb = qkvpool.tile([DL, s], BF16, tag="vt")
                for wsb, dst, is_v in (
                    (wq_sb, q_rope, False),
                    (wk_sb, k_rope, False),
                    (wv_sb, vt_sb, True),
                ):
                    for ch in range(nch):
                        cols = slice(ch * chunk, (ch + 1) * chunk)
                        ps = psA.tile([128, chunk], F32, tag="sc0")
                        for dt in range(dt8):
                            nc.tensor.matmul(
                                ps[:],
                                wsb[:, dt, :],
                                xt_sb[:, dt, cols],
                                start=(dt == 0),
                                stop=(dt == dt8 - 1),
                            )
                        if is_v:
                            nc.scalar.copy(dst[:, cols], ps[:])
                        else:
                            tsb = ropepool.tile([128, chunk], BF16, tag="tsb")
                            nc.scalar.copy(tsb[:], ps[:])
                            rps = psA.tile([128, chunk], F32, tag="sc1")
                            nc.tensor.matmul(
                                rps[:], mp_sb[:], tsb[:], start=True, stop=True
                            )
                            m1 = ropepool.tile([128, chunk], BF16, tag="m1")
                            nc.vector.tensor_tensor(
                                m1[:], ps[:], cs128[:, cols], mybir.AluOpType.mult
                            )
                            m2 = ropepool.tile([128, chunk], BF16, tag="m2")
                            nc.vector.tensor_tensor(
                                m2[:], rps[:], sn128[:, cols], mybir.AluOpType.mult
                            )
                            nc.vector.tensor_tensor(
                                dst[:, cols], m1[:], m2[:], mybir.AluOpType.add
                            )

                if debug:
                    for name, tl in (("dbg_q", q_rope), ("dbg_k", k_rope), ("dbg_v", vt_sb)):
                        df = outpool.tile([DL, s], F32, tag="dbgf")
                        nc.vector.tensor_copy(df[:], tl[:])
                        nc.sync.dma_start(
                            {"dbg_q": dbg_q, "dbg_k": dbg_k, "dbg_v": dbg_v}[name][bi],
                            df[:],
                        )

                # v_aug [s_k, hd+1] blocks (ones column fuses the softmax sum)
                v_sb = qkvpool.tile([128, kt, HL, HD + 1], BF16, tag="v_sb")
                nc.vector.memset(v_sb[:, :, :, HD : HD + 1], 1.0)
                # transpose v^T -> v via identity matmul; the psum->sbuf
                # copies scatter the two head halves into the v_aug layout.
                for kt4 in range(kt // 4):
                    vps = psC.tile([128, 4, 128], F32, tag="tp")
                    for j in range(4):
                        ktt = kt4 * 4 + j
                        nc.tensor.matmul(
                            vps[:, j, :],
                            vt_sb[:, ktt * 128 : (ktt + 1) * 128],
                            id_sb[:],
                            start=True,
                            stop=True,
                        )
                    for h in range(HL):
                        nc.vector.tensor_copy(
                            v_sb[:, kt4 * 4 : (kt4 + 1) * 4, h, 0:HD],
                            vps[:, :, h * HD : (h + 1) * HD],
                        )

                if bi == 0:
                    # stage Wo now: overlaps batch-0 attention / batch-1 QKV
                    for dt in range(dt8):
                        wof = stpool.tile([128, D], F32, tag="wostage")
                        nc.sync.dma_start(wof[:], wo[dt * 128 : (dt + 1) * 128, :])
                        nc.scalar.copy(wo_sb[:, dt, :], wof[:])

                # ---- attention: transposed scores -> exp -> PV (+sigma) ----
                for ch in range(nch):
                    cols = slice(ch * chunk, (ch + 1) * chunk)
                    pts = {}
                    for gi, (k0, glen) in enumerate(groups):
                        for h in range(HL):
                            rows = slice(h * HD, (h + 1) * HD)
                            sg = psA.tile([128, GMAX, chunk], F32, tag=f"sc{h}")
                            for j in range(glen):
                                ktt = k0 + j
                                nc.tensor.matmul(
                                    sg[:, j, :],
                                    k_rope[rows, ktt * 128 : (ktt + 1) * 128],
                                    q_rope[rows, cols],
                                    start=True,
                                    stop=True,
                                )
                            pt = ptpool.tile([128, GMAX, chunk], BF16, tag="pt")
                            nc.scalar.activation(
                                pt[:, :glen, :],
                                sg[:, :glen, :],
                                mybir.ActivationFunctionType.Exp,
                                bias=biasc[:],
                                scale=EXP_SCALE,
                            )
                            pts[(gi, h)] = pt
                            if debug and bi == 0 and ch == 0 and gi == 0:
                                dsc = outpool.tile([128, chunk], F32, tag="dbgsc")
                                nc.vector.tensor_copy(dsc[:], sg[:, 0, :])
                                nc.sync.dma_start(dbg_sc[h], dsc[:])
                                dpt = outpool.tile([128, chunk], F32, tag="dbgpt")
                                nc.vector.tensor_copy(dpt[:], pt[:, 0, :])
                                nc.sync.dma_start(dbg_pt[h], dpt[:])
                    for h in range(HL):
                        pv = psB.tile([HD + 1, chunk], F32, tag="pv")
                        for gi, (k0, glen) in enumerate(groups):
                            pt = pts[(gi, h)]
                            for j in range(glen):
                                ktt = k0 + j
                                nc.tensor.matmul(
                                    pv[:],
                                    v_sb[:, ktt, h, :],
                                    pt[:, j, :],
                                    start=(ktt == 0),
                                    stop=(ktt == kt - 1),
                                )
                        if debug and bi == 0 and ch == 0:
                            dpv = outpool.tile([HD + 1, chunk], F32, tag="dbgpv")
                            nc.vector.tensor_copy(dpv[:], pv[:])
                            nc.sync.dma_start(dbg_pv[h], dpv[:])
                        # 1/sigma: fast reciprocal of the ones-column row, then
                        # log2-doubling DMA broadcast down the partitions
                        # (on the gpsimd queue to keep the sync queue free).
                        bcast = sigpool.tile([HD + 1, chunk], F32, tag="bcast")
                        nc.vector.reciprocal(
                            bcast[HD : HD + 1, :], pv[HD : HD + 1, :]
                        )
                        nc.gpsimd.dma_start(bcast[0:1, :], bcast[HD : HD + 1, :])
                        kk = 1
                        while kk < HD:
                            nc.gpsimd.dma_start(bcast[kk : 2 * kk, :], bcast[0:kk, :])
                            kk *= 2
                        oh = sigpool.tile([HD, chunk], BF16, tag="oh")
                        nc.vector.tensor_tensor(
                            oh[:], pv[0:HD, :], bcast[0:HD, :], mybir.AluOpType.mult
                        )
                        nc.gpsimd.dma_start(
                            attnout[
                                h * HD : (h + 1) * HD,
                                bi * s + ch * chunk : bi * s + (ch + 1) * chunk,
                            ],
                            oh[:],
                        )

            if debug:
                daf = outpool.tile([DL, b * s], F32, tag="dbga")
                nc.vector.tensor_copy(daf[:], attnout[:])
                nc.sync.dma_start(dbg_att[:], daf[:])

            # ---- AllToAll: attnout^T [128, b*s] -> per-shard [1024, shard] ----
            a2a_in = dram.tile([N_CORES, DL, shard], BF16, tag="a2a_in")
            a2a_out = dram.tile([N_CORES, DL, shard], BF16, tag="a2a_out")
            for j in range(N_CORES):
                nc.sync.dma_start(a2a_in[j], attnout[:, j * shard : (j + 1) * shard])
            nc.gpsimd.collective_compute(
                "AllToAll",
                mybir.AluOpType.bypass,
                replica_groups=[list(range(N_CORES))],
                ins=[a2a_in.opt()],
                outs=[a2a_out.opt()],
            )
            recv = cpool.tile([DL, N_CORES, shard], BF16, tag="recv")
            for i in range(N_CORES):
                nc.sync.dma_start(recv[:, i, :], a2a_out[i])

            # ---- output projection for this core's shard ----
            for j in range(shard // 128):
                for nco in range(D // 512):
                    wps = psA.tile([128, 512], F32, tag=f"sc{(j * 2 + nco) % 2}")
                    for i in range(N_CORES):
                        nc.tensor.matmul(
                            wps[:],
                            recv[:, i, j * 128 : (j + 1) * 128],
                            wo_sb[:, i, nco * 512 : (nco + 1) * 512],
                            start=(i == 0),
                            stop=(i == N_CORES - 1),
                        )
                    osb = outpool.tile([128, 512], F32, tag="osb")
                    nc.vector.tensor_copy(osb[:], wps[:])
                    nc.sync.dma_start(
                        out[j * 128 : (j + 1) * 128, nco * 512 : (nco + 1) * 512],
                        osb[:],
                    )

    split_excess_waits(nc)
    return nc


def make_in_maps(x, cos, sin, Wq, Wk, Wv, Wo, b, s):
    x = np.ascontiguousarray(x, dtype=np.float32)
    in_maps = []
    for c in range(N_CORES):
        cs = slice(c * DL, (c + 1) * DL)
        in_maps.append(
            {
                "x": x,
                "cos": np.ascontiguousarray(cos, dtype=np.float32),
                "sin": np.ascontiguousarray(sin, dtype=np.float32),
                "wq": np.ascontiguousarray(Wq[:, cs], dtype=np.float32),
                "wk": np.ascontiguousarray(Wk[:, cs], dtype=np.float32),
                "wv": np.ascontiguousarray(Wv[:, cs], dtype=np.float32),
                "wo": np.ascontiguousarray(Wo, dtype=np.float32),
            }
        )
    return in_maps


_NC_CACHE = {}


def run(x, cos, sin, Wq, Wk, Wv, Wo, trace=False, chunk=512, pt_bufs=18):
    b, s, _ = x.shape
    key = (b, s, chunk, pt_bufs)
    if key not in _NC_CACHE:
        _NC_CACHE[key] = build_nc(b=b, s=s, chunk=chunk, pt_bufs=pt_bufs)
    nc = _NC_CACHE[key]
    in_maps = make_in_maps(x, cos, sin, Wq, Wk, Wv, Wo, b, s)
    res = run_bass_kernel_spmd(nc, in_maps, list(range(N_CORES)), trace=trace)
    shard = b * s // N_CORES
    full = np.concatenate([res.results[c]["out"] for c in range(N_CORES)], axis=0)
    return full.reshape(b, s, D), res


def kernel(x, cos, sin, Wq, Wk, Wv, Wo):
    out, _ = run(
        np.asarray(x), np.asarray(cos), np.asarray(sin),
        np.asarray(Wq), np.asarray(Wk), np.asarray(Wv), np.asarray(Wo),
    )
    return out.astype(np.float32)


# revision 42
# speedup vs baseline: 1.7944x; 1.0155x over previous
"""Multi-head attention with RoPE on 8 Trainium2 NeuronCores.

Problem: x[2,2048,1024] -> MHA(16 heads, hd=64, NeoX RoPE, non-causal) -> out.

Sharding: tensor-parallel over heads. Each core owns 2 heads:
  - computes q^T,k^T (RoPE'd) and v for its heads over the full sequence
    (weights column-sliced on host; x replicated),
  - flash-style attention entirely on-chip with *transposed* scores
    [s_k, s_q] so the softmax denominator comes from a fused ones-column
    in V (no P transpose, no row-max pass; a constant bias inside the exp
    activation keeps the range safe),
  - one small AllToAll redistributes attention outputs so each core holds
    all 1024 attn dims for its 512-row output shard,
  - local Wo matmul produces the shard; host concatenates shards.

All matmuls run in bf16 (fp32 PSUM accumulation); rel-err tolerance is 2e-2.
"""

import sys

sys.path.insert(0, "/opt/trn_rl_repo")

import numpy as np  # noqa: E402

import concourse.bass as bass  # noqa: E402
import concourse.mybir as mybir  # noqa: E402
import concourse.tile as tile  # noqa: E402
from concourse.bass_utils import run_bass_kernel_spmd  # noqa: E402

N_CORES = 8
D = 1024
H = 16
HD = 64
HL = H // N_CORES  # local heads per core
DL = HL * HD  # 128 local attn dims
EXP_SCALE = 0.125  # 1/sqrt(hd)
EXP_BIAS = -24.0  # exp(s/8 - 24): cancels in softmax, keeps fp32 range safe
GMAX = 2  # scores-psum banks per (head, kt-group); 2*GMAX + 2 PV + 2 transpose <= 8

F32 = mybir.dt.float32
BF16 = mybir.dt.bfloat16


def _kt_groups(kt):
    """Split kt score-tiles into groups of <=GMAX (wider exp instructions)."""
    groups = []
    k0 = 0
    while k0 < kt:
        n3 = (kt - k0) // GMAX
        g = GMAX if n3 > 0 and (kt - k0) % GMAX != 1 else min(GMAX - 1, kt - k0)
        if (kt - k0) % GMAX == 0:
            g = GMAX
        groups.append((k0, g))
        k0 += g
    return groups


def _perm_matrix():
    """lhsT for the rotate_half matmul: qrot^T = lhsT.T @ q^T.

    Per head block at offset o: rot(q)[i] = -q[i+32] for i<32,
    rot(q)[i] = q[i-32] for 32<=i<64.
    """
    mt = np.zeros((DL, DL), dtype=np.float32)
    for o in (0, HD):
        for r in range(HD // 2):
            mt[o + r, o + r + HD // 2] = -1.0
            mt[o + r + HD // 2, o + r] = 1.0
    return np.ascontiguousarray(mt.T)


def split_excess_waits(nc, max_waits=1):
    """This container's walrus rejects >1 semaphore wait per instruction;
    split excess waits onto NoOp carriers on the same engine."""
    for bb in nc.m.functions[0].blocks:
        insts = bb.instructions
        idx = 0
        while idx < len(insts):
            ins = insts[idx]
            si = ins.sync_info
            if si is not None and si.on_wait and len(si.on_wait) > max_waits:
                ow = list(si.on_wait)
                si.on_wait = ow[-max_waits:]
                extra = ow[:-max_waits]
                k = 0
                while extra:
                    chunk, extra = extra[:max_waits], extra[max_waits:]
                    c = mybir.InstNoOp(name=f"{ins.name}-ws{k}", ins=[], outs=[])
                    c.engine = ins.engine
                    c.sync_info = mybir.SyncInfo(on_wait=chunk, on_update=[])
                    nc.register_instruction(c)
                    insts.insert(idx, c)
                    idx += 1
                    k += 1
            idx += 1


def build_nc(b=2, s=2048, chunk=512, pt_bufs=18, debug=False):
    kt = s // 128  # score tiles along s_k per batch
    nch = s // chunk  # s_q chunks per batch
    shard = b * s // N_CORES  # output rows per core
    groups = _kt_groups(kt)
    dt8 = D // 128  # contraction tiles for projections

    nc = bass.Bass()
    x = nc.declare_dram_parameter("x", [b, s, D], F32, isOutput=False)
    cosp = nc.declare_dram_parameter("cos", [s, HD // 2], F32, isOutput=False)
    sinp = nc.declare_dram_parameter("sin", [s, HD // 2], F32, isOutput=False)
    wq = nc.declare_dram_parameter("wq", [D, DL], F32, isOutput=False)
    wk = nc.declare_dram_parameter("wk", [D, DL], F32, isOutput=False)
    wv = nc.declare_dram_parameter("wv", [D, DL], F32, isOutput=False)
    wo = nc.declare_dram_parameter("wo", [D, D], F32, isOutput=False)
    out = nc.declare_dram_parameter("out", [shard, D], F32, isOutput=True)
    if debug:
        dbg_q = nc.declare_dram_parameter("dbg_q", [b, DL, s], F32, isOutput=True)
        dbg_k = nc.declare_dram_parameter("dbg_k", [b, DL, s], F32, isOutput=True)
        dbg_v = nc.declare_dram_parameter("dbg_v", [b, DL, s], F32, isOutput=True)
        dbg_att = nc.declare_dram_parameter("dbg_att", [DL, b * s], F32, isOutput=True)
        dbg_sc = nc.declare_dram_parameter("dbg_sc", [HL, 128, chunk], F32, isOutput=True)
        dbg_pt = nc.declare_dram_parameter("dbg_pt", [HL, 128, chunk], F32, isOutput=True)
        dbg_pv = nc.declare_dram_parameter("dbg_pv", [HL, HD + 1, chunk], F32, isOutput=True)

    mperm = nc.inline_tensor(_perm_matrix().astype(np.float32), name="mperm")
    ident = nc.inline_tensor(np.eye(128, dtype=np.float32), name="ident128")

    with tile.TileContext(nc) as tc:
        with (
            tc.tile_pool(name="dram", bufs=1, space="DRAM") as dram,
            tc.tile_pool(name="const", bufs=1) as cpool,
            tc.tile_pool(name="stage", bufs=1) as stpool,
            tc.tile_pool(name="xin", bufs=3) as xpool,
            tc.tile_pool(name="xbf", bufs=3) as xbpool,
            tc.tile_pool(name="xt", bufs=1) as xtpool,
            tc.tile_pool(name="qkv", bufs=2) as qkvpool,
            tc.tile_pool(name="rope", bufs=3) as ropepool,
            tc.tile_pool(name="pt", bufs=pt_bufs) as ptpool,
            tc.tile_pool(name="att", bufs=1) as attpool,
            tc.tile_pool(name="sig", bufs=4) as sigpool,
            tc.tile_pool(name="outp", bufs=2) as outpool,
            # PSUM is 8 banks total and pool slots are static:
            # psA holds tags "sc0"/"sc1" ([128, GMAX*chunk] = 3 banks each, 6
            # total; projection/rot/Wo accumulators borrow these tags), psB
            # holds 2 PV accumulators (1 bank each). 6 + 2 = 8.
            tc.tile_pool(name="psA", bufs=1, space="PSUM") as psA,
            tc.tile_pool(name="psB", bufs=2, space="PSUM") as psB,
            tc.tile_pool(name="psC", bufs=2, space="PSUM") as psC,
        ):
            # ---- constants: weights (bf16), rotation matrix, cos/sin rows ----
            wq_sb = cpool.tile([128, dt8, DL], BF16, tag="wq")
            wk_sb = cpool.tile([128, dt8, DL], BF16, tag="wk")
            wv_sb = cpool.tile([128, dt8, DL], BF16, tag="wv")
            for wparam, wsb in ((wq, wq_sb), (wk, wk_sb), (wv, wv_sb)):
                wf = stpool.tile([128, dt8, DL], F32, tag="wstage")
                nc.sync.dma_start(wf[:], wparam.rearrange("(t p) m -> p t m", p=128))
                nc.scalar.copy(wsb[:], wf[:])

            mp_f = stpool.tile([DL, DL], F32, tag="mperm_f")
            nc.sync.dma_start(mp_f[:], mperm[:])
            mp_sb = cpool.tile([DL, DL], BF16, tag="mperm")
            nc.vector.tensor_copy(mp_sb[:], mp_f[:])

            id_f = stpool.tile([128, 128], F32, tag="ident_f")
            nc.sync.dma_start(id_f[:], ident[:])
            id_sb = cpool.tile([128, 128], BF16, tag="ident")
            nc.vector.tensor_copy(id_sb[:], id_f[:])

            # cos/sin: [s, 32] -> transposed, doubled rows -> [128, s] bf16
            st16 = s // 128
            cs128 = cpool.tile([128, s], BF16, tag="cs")
            sn128 = cpool.tile([128, s], BF16, tag="sn")
            for p, t128 in ((cosp, cs128), (sinp, sn128)):
                cf = stpool.tile([128, st16, HD // 2], F32, tag="cstage")
                nc.sync.dma_start(cf[:], p.rearrange("(t p) d -> p t d", p=128))
                cb = stpool.tile([128, st16, HD // 2], BF16, tag="cstage_b")
                nc.vector.tensor_copy(cb[:], cf[:])
                # XBAR transpose needs 128-divisible tiles: transpose 4
                # s-tiles (4*32 = 128 free) at once, then scatter row-blocks.
                for blk in range(st16 // 4):
                    ctmp = stpool.tile([128, 128], BF16, tag="cs_t")
                    nc.sync.dma_start_transpose(
                        out=ctmp[:], in_=cb[:, blk * 4 : (blk + 1) * 4, :]
                    )
                    for j in range(4):
                        st = blk * 4 + j
                        nc.sync.dma_start(
                            t128[0:32, st * 128 : (st + 1) * 128],
                            ctmp[j * 32 : (j + 1) * 32, :],
                        )
                for r in (32, 64, 96):
                    nc.sync.dma_start(t128[r : r + 32, :], t128[0:32, :])

            biasc = cpool.tile([128, 1], F32, tag="biasc")
            nc.vector.memset(biasc[:], EXP_BIAS)

            attnout = attpool.tile([DL, b * s], BF16, tag="attnout")

            # Wo staging is off the startup critical path: emit after batch 0's
            # x pipeline in program order (scheduler fills DMA gaps with it).
            wo_sb = cpool.tile([128, dt8, D], BF16, tag="wo")

            for bi in range(b):
                # ---- x^T (bf16) via cast + DMA transpose ----
                xt_sb = xtpool.tile([128, dt8, s], BF16, tag="xt")
                for st in range(st16):
                    xf = xpool.tile([128, D], F32, tag="xf")
                    nc.sync.dma_start(xf[:], x[bi, st * 128 : (st + 1) * 128, :])
                    xb_ = xbpool.tile([128, D], BF16, tag="xb")
                    nc.vector.tensor_copy(xb_[:], xf[:])
                    # transpose via identity matmul: out = x_tile.T @ I.
                    # 4 transposes share one psum bank -> 1 batched copy.
                    for dt4 in range(dt8 // 4):
                        tps = psC.tile([128, 4, 128], F32, tag="tp")
                        for j in range(4):
                            dt = dt4 * 4 + j
                            nc.tensor.matmul(
                                tps[:, j, :],
                                xb_[:, dt * 128 : (dt + 1) * 128],
                                id_sb[:],
                                start=True,
                                stop=True,
                            )
                        nc.vector.tensor_copy(
                            xt_sb[:, dt4 * 4 : (dt4 + 1) * 4, st * 128 : (st + 1) * 128],
                            tps[:],
                        )

                # ---- q,k projections + RoPE; v projection + transpose ----
                q_rope = qkvpool.tile([DL, s], BF16, tag="q_rope")
                k_rope = qkvpool.tile([DL, s], BF16, tag="k_rope")
                vt_sb = qkvpool.tile([DL, s], BF16, tag="vt")
                for wsb, dst, is_v in (
                    (wq_sb, q_rope, False),
                    (wk_sb, k_rope, False),
                    (wv_sb, vt_sb, True),
                ):
                    for ch in range(nch):
                        cols = slice(ch * chunk, (ch + 1) * chunk)
                        ps = psA.tile([128, chunk], F32, tag=f"sc{ch % 2}")
                        for dt in range(dt8):
                            nc.tensor.matmul(
                                ps[:],
                                wsb[:, dt, :],
                                xt_sb[:, dt, cols],
                                start=(dt == 0),
                                stop=(dt == dt8 - 1),
                            )
                        if is_v:
                            nc.scalar.copy(dst[:, cols], ps[:])
                        else:
                            tsb = ropepool.tile([128, chunk], BF16, tag="tsb")
                            nc.scalar.copy(tsb[:], ps[:])
                            rps = psC.tile([128, chunk], F32, tag="tp")
                            nc.tensor.matmul(
                                rps[:], mp_sb[:], tsb[:], start=True, stop=True
                            )
                            m1 = ropepool.tile([128, chunk], BF16, tag="m1")
                            nc.vector.tensor_tensor(
                                m1[:], ps[:], cs128[:, cols], mybir.AluOpType.mult
                            )
                            m2 = ropepool.tile([128, chunk], BF16, tag="m2")
                            nc.vector.tensor_tensor(
                                m2[:], rps[:], sn128[:, cols], mybir.AluOpType.mult
                            )
                            nc.vector.tensor_tensor(
                                dst[:, cols], m1[:], m2[:], mybir.AluOpType.add
                            )

                if debug:
                    for name, tl in (("dbg_q", q_rope), ("dbg_k", k_rope), ("dbg_v", vt_sb)):
                        df = outpool.tile([DL, s], F32, tag="dbgf")
                        nc.vector.tensor_copy(df[:], tl[:])
                        nc.sync.dma_start(
                            {"dbg_q": dbg_q, "dbg_k": dbg_k, "dbg_v": dbg_v}[name][bi],
                            df[:],
                        )

                # v_aug [s_k, hd+1] blocks (ones column fuses the softmax sum)
                v_sb = qkvpool.tile([128, kt, HL, HD + 1], BF16, tag="v_sb")
                nc.vector.memset(v_sb[:, :, :, HD : HD + 1], 1.0)
                # transpose v^T -> v via identity matmul; the psum->sbuf
                # copies scatter the two head halves into the v_aug layout.
                for kt4 in range(kt // 4):
                    vps = psC.tile([128, 4, 128], F32, tag="tp")
                    for j in range(4):
                        ktt = kt4 * 4 + j
                        nc.tensor.matmul(
                            vps[:, j, :],
                            vt_sb[:, ktt * 128 : (ktt + 1) * 128],
                            id_sb[:],
                            start=True,
                            stop=True,
                        )
                    for h in range(HL):
                        nc.vector.tensor_copy(
                            v_sb[:, kt4 * 4 : (kt4 + 1) * 4, h, 0:HD],
                            vps[:, :, h * HD : (h + 1) * HD],
                        )

                if bi == 0:
                    # stage Wo now: overlaps batch-0 attention / batch-1 QKV
                    for dt in range(dt8):
                        wof = stpool.tile([128, D], F32, tag="wostage")
                        nc.sync.dma_start(wof[:], wo[dt * 128 : (dt + 1) * 128, :])
                        nc.scalar.copy(wo_sb[:, dt, :], wof[:])

                # ---- attention: transposed scores -> exp -> PV (+sigma) ----
                for ch in range(nch):
                    cols = slice(ch * chunk, (ch + 1) * chunk)
                    pts = {}
                    for gi, (k0, glen) in enumerate(groups):
                        for h in range(HL):
                            rows = slice(h * HD, (h + 1) * HD)
                            sg = psA.tile([128, GMAX, chunk], F32, tag=f"sc{h}")
                            for j in range(glen):
                                ktt = k0 + j
                                nc.tensor.matmul(
                                    sg[:, j, :],
                                    k_rope[rows, ktt * 128 : (ktt + 1) * 128],
                                    q_rope[rows, cols],
                                    start=True,
                                    stop=True,
                                )
                            pt = ptpool.tile([128, GMAX, chunk], BF16, tag="pt")
                            nc.scalar.activation(
                                pt[:, :glen, :],
                                sg[:, :glen, :],
                                mybir.ActivationFunctionType.Exp,
                                bias=biasc[:],
                                scale=EXP_SCALE,
                            )
                            pts[(gi, h)] = pt
                            if debug and bi == 0 and ch == 0 and gi == 0:
                                dsc = outpool.tile([128, chunk], F32, tag="dbgsc")
                                nc.vector.tensor_copy(dsc[:], sg[:, 0, :])
                                nc.sync.dma_start(dbg_sc[h], dsc[:])
                                dpt = outpool.tile([128, chunk], F32, tag="dbgpt")
                                nc.vector.tensor_copy(dpt[:], pt[:, 0, :])
                                nc.sync.dma_start(dbg_pt[h], dpt[:])
                    for h in range(HL):
                        pv = psB.tile([HD + 1, chunk], F32, tag="pv")
                        for gi, (k0, glen) in enumerate(groups):
                            pt = pts[(gi, h)]
                            for j in range(glen):
                                ktt = k0 + j
                                nc.tensor.matmul(
                                    pv[:],
                                    v_sb[:, ktt, h, :],
                                    pt[:, j, :],
                                    start=(ktt == 0),
                                    stop=(ktt == kt - 1),
                                )
                        if debug and bi == 0 and ch == 0:
                            dpv = outpool.tile([HD + 1, chunk], F32, tag="dbgpv")
                            nc.vector.tensor_copy(dpv[:], pv[:])
                            nc.sync.dma_start(dbg_pv[h], dpv[:])
                        # 1/sigma: fast reciprocal of the ones-column row, then
                        # log2-doubling DMA broadcast down the partitions
                        # (on the gpsimd queue to keep the sync queue free).
                        bcast = sigpool.tile([HD + 1, chunk], F32, tag="bcast")
                        nc.vector.reciprocal(
                            bcast[HD : HD + 1, :], pv[HD : HD + 1, :]
                        )
                        nc.gpsimd.dma_start(bcast[0:1, :], bcast[HD : HD + 1, :])
                        kk = 1
                        while kk < HD:
                            nc.gpsimd.dma_start(bcast[kk : 2 * kk, :], bcast[0:kk, :])
                            kk *= 2
                        oh = sigpool.tile([HD, chunk], BF16, tag="oh")
                        nc.vector.tensor_tensor(
                            oh[:], pv[0:HD, :], bcast[0:HD, :], mybir.AluOpType.mult
                        )
                        nc.gpsimd.dma_start(
                            attnout[
                                h * HD : (h + 1) * HD,
                                bi * s + ch * chunk : bi * s + (ch + 1) * chunk,
                            ],
                            oh[:],
                        )

            if debug:
                daf = outpool.tile([DL, b * s], F32, tag="dbga")
                nc.vector.tensor_copy(daf[:], attnout[:])
                nc.sync.dma_start(dbg_att[:], daf[:])

            # ---- AllToAll: attnout^T [128, b*s] -> per-shard [1024, shard] ----
            a2a_in = dram.tile([N_CORES, DL, shard], BF16, tag="a2a_in")
            a2a_out = dram.tile([N_CORES, DL, shard], BF16, tag="a2a_out")
            for j in range(N_CORES):
                nc.sync.dma_start(a2a_in[j], attnout[:, j * shard : (j + 1) * shard])
            nc.gpsimd.collective_compute(
                "AllToAll",
                mybir.AluOpType.bypass,
                replica_groups=[list(range(N_CORES))],
                ins=[a2a_in.opt()],
                outs=[a2a_out.opt()],
            )
            recv = cpool.tile([DL, N_CORES, shard], BF16, tag="recv")
            for i in range(N_CORES):
                nc.sync.dma_start(recv[:, i, :], a2a_out[i])

            # ---- output projection for this core's shard ----
            for j in range(shard // 128):
                for nco in range(D // 512):
                    wps = psA.tile([128, 512], F32, tag=f"sc{(j * 2 + nco) % 2}")
                    for i in range(N_CORES):
                        nc.tensor.matmul(
                            wps[:],
                            recv[:, i, j * 128 : (j + 1) * 128],
                            wo_sb[:, i, nco * 512 : (nco + 1) * 512],
                            start=(i == 0),
                            stop=(i == N_CORES - 1),
                        )
                    osb = outpool.tile([128, 512], F32, tag="osb")
                    nc.vector.tensor_copy(osb[:], wps[:])
                    nc.sync.dma_start(
                        out[j * 128 : (j + 1) * 128, nco * 512 : (nco + 1) * 512],
                        osb[:],
                    )

    split_excess_waits(nc)
    return nc


def make_in_maps(x, cos, sin, Wq, Wk, Wv, Wo, b, s):
    x = np.ascontiguousarray(x, dtype=np.float32)
    in_maps = []
    for c in range(N_CORES):
        cs = slice(c * DL, (c + 1) * DL)
        in_maps.append(
            {
                "x": x,
                "cos": np.ascontiguousarray(cos, dtype=np.float32),
                "sin": np.ascontiguousarray(sin, dtype=np.float32),
                "wq": np.ascontiguousarray(Wq[:, cs], dtype=np.float32),
                "wk": np.ascontiguousarray(Wk[:, cs], dtype=np.float32),
                "wv": np.ascontiguousarray(Wv[:, cs], dtype=np.float32),
                "wo": np.ascontiguousarray(Wo, dtype=np.float32),
            }
        )
    return in_maps


_NC_CACHE = {}


def run(x, cos, sin, Wq, Wk, Wv, Wo, trace=False, chunk=512, pt_bufs=18):
    b, s, _ = x.shape
    key = (b, s, chunk, pt_bufs)
    if key not in _NC_CACHE:
        _NC_CACHE[key] = build_nc(b=b, s=s, chunk=chunk, pt_bufs=pt_bufs)
    nc = _NC_CACHE[key]
    in_maps = make_in_maps(x, cos, sin, Wq, Wk, Wv, Wo, b, s)
    res = run_bass_kernel_spmd(nc, in_maps, list(range(N_CORES)), trace=trace)
    shard = b * s // N_CORES
    full = np.concatenate([res.results[c]["out"] for c in range(N_CORES)], axis=0)
    return full.reshape(b, s, D), res


def kernel(x, cos, sin, Wq, Wk, Wv, Wo):
    out, _ = run(
        np.asarray(x), np.asarray(cos), np.asarray(sin),
        np.asarray(Wq), np.asarray(Wk), np.asarray(Wv), np.asarray(Wo),
    )
    return out.astype(np.float32)


# revision 51
# speedup vs baseline: 1.9296x; 1.0754x over previous
"""Multi-head attention with RoPE on 8 Trainium2 NeuronCores.

Problem: x[2,2048,1024] -> MHA(16 heads, hd=64, NeoX RoPE, non-causal) -> out.

Sharding: tensor-parallel over heads. Each core owns 2 heads:
  - computes q^T,k^T (RoPE'd) and v for its heads over the full sequence
    (weights column-sliced on host; x replicated),
  - flash-style attention entirely on-chip with *transposed* scores
    [s_k, s_q] so the softmax denominator comes from a fused ones-column
    in V (no P transpose, no row-max pass; a constant bias inside the exp
    activation keeps the range safe),
  - one small AllToAll redistributes attention outputs so each core holds
    all 1024 attn dims for its 512-row output shard,
  - local Wo matmul produces the shard; host concatenates shards.

All matmuls run in bf16 (fp32 PSUM accumulation); rel-err tolerance is 2e-2.
"""

import sys

sys.path.insert(0, "/opt/trn_rl_repo")

import numpy as np  # noqa: E402

import concourse.bass as bass  # noqa: E402
import concourse.mybir as mybir  # noqa: E402
import concourse.tile as tile  # noqa: E402
from concourse.bass_utils import run_bass_kernel_spmd  # noqa: E402

N_CORES = 8
D = 1024
H = 16
HD = 64
HL = H // N_CORES  # local heads per core
DL = HL * HD  # 128 local attn dims
EXP_SCALE = 0.125  # 1/sqrt(hd)
EXP_BIAS = -24.0  # exp(s/8 - 24): cancels in softmax, keeps fp32 range safe
GMAX = 2  # scores-psum banks per (head, kt-group); 2*GMAX + 2 PV + 2 transpose <= 8

F32 = mybir.dt.float32
BF16 = mybir.dt.bfloat16


def _kt_groups(kt):
    """Split kt score-tiles into groups of <=GMAX (wider exp instructions)."""
    groups = []
    k0 = 0
    while k0 < kt:
        n3 = (kt - k0) // GMAX
        g = GMAX if n3 > 0 and (kt - k0) % GMAX != 1 else min(GMAX - 1, kt - k0)
        if (kt - k0) % GMAX == 0:
            g = GMAX
        groups.append((k0, g))
        k0 += g
    return groups


def _perm_matrix():
    """lhsT for the rotate_half matmul: qrot^T = lhsT.T @ q^T.

    Per head block at offset o: rot(q)[i] = -q[i+32] for i<32,
    rot(q)[i] = q[i-32] for 32<=i<64.
    """
    mt = np.zeros((DL, DL), dtype=np.float32)
    for o in (0, HD):
        for r in range(HD // 2):
            mt[o + r, o + r + HD // 2] = -1.0
            mt[o + r + HD // 2, o + r] = 1.0
    return np.ascontiguousarray(mt.T)


def split_excess_waits(nc, max_waits=1):
    """This container's walrus rejects >1 semaphore wait per instruction;
    split excess waits onto NoOp carriers on the same engine."""
    for bb in nc.m.functions[0].blocks:
        insts = bb.instructions
        idx = 0
        while idx < len(insts):
            ins = insts[idx]
            si = ins.sync_info
            if si is not None and si.on_wait and len(si.on_wait) > max_waits:
                ow = list(si.on_wait)
                si.on_wait = ow[-max_waits:]
                extra = ow[:-max_waits]
                k = 0
                while extra:
                    chunk, extra = extra[:max_waits], extra[max_waits:]
                    c = mybir.InstNoOp(name=f"{ins.name}-ws{k}", ins=[], outs=[])
                    c.engine = ins.engine
                    c.sync_info = mybir.SyncInfo(on_wait=chunk, on_update=[])
                    nc.register_instruction(c)
                    insts.insert(idx, c)
                    idx += 1
                    k += 1
            idx += 1


def build_nc(b=2, s=2048, chunk=512, pt_bufs=16, debug=False):
    kt = s // 128  # score tiles along s_k per batch
    nch = s // chunk  # s_q chunks per batch
    shard = b * s // N_CORES  # output rows per core
    groups = _kt_groups(kt)
    dt8 = D // 128  # contraction tiles for projections

    nc = bass.Bass()
    x = nc.declare_dram_parameter("x", [b, s, D], F32, isOutput=False)
    cosp = nc.declare_dram_parameter("cos", [s, HD // 2], F32, isOutput=False)
    sinp = nc.declare_dram_parameter("sin", [s, HD // 2], F32, isOutput=False)
    wq = nc.declare_dram_parameter("wq", [D, DL], F32, isOutput=False)
    wk = nc.declare_dram_parameter("wk", [D, DL], F32, isOutput=False)
    wv = nc.declare_dram_parameter("wv", [D, DL], F32, isOutput=False)
    wo = nc.declare_dram_parameter("wo", [D, D], F32, isOutput=False)
    out = nc.declare_dram_parameter("out", [shard, D], F32, isOutput=True)
    if debug:
        dbg_q = nc.declare_dram_parameter("dbg_q", [b, DL, s], F32, isOutput=True)
        dbg_k = nc.declare_dram_parameter("dbg_k", [b, DL, s], F32, isOutput=True)
        dbg_v = nc.declare_dram_parameter("dbg_v", [b, DL, s], F32, isOutput=True)
        dbg_att = nc.declare_dram_parameter("dbg_att", [DL, b * s], F32, isOutput=True)
        dbg_sc = nc.declare_dram_parameter("dbg_sc", [HL, 128, chunk], F32, isOutput=True)
        dbg_pt = nc.declare_dram_parameter("dbg_pt", [HL, 128, chunk], F32, isOutput=True)
        dbg_pv = nc.declare_dram_parameter("dbg_pv", [HL, HD + 1, chunk], F32, isOutput=True)

    mperm = nc.inline_tensor(_perm_matrix().astype(np.float32), name="mperm")
    ident = nc.inline_tensor(np.eye(128, dtype=np.float32), name="ident128")

    with tile.TileContext(nc) as tc:
        with (
            tc.tile_pool(name="dram", bufs=1, space="DRAM") as dram,
            tc.tile_pool(name="const", bufs=1) as cpool,
            tc.tile_pool(name="stage", bufs=1) as stpool,
            tc.tile_pool(name="xin", bufs=3) as xpool,
            tc.tile_pool(name="xbf", bufs=3) as xbpool,
            tc.tile_pool(name="xt", bufs=1) as xtpool,
            tc.tile_pool(name="qkv", bufs=2) as qkvpool,
            tc.tile_pool(name="rope", bufs=2) as ropepool,
            tc.tile_pool(name="pt", bufs=pt_bufs) as ptpool,
            tc.tile_pool(name="att", bufs=1) as attpool,
            tc.tile_pool(name="sig", bufs=2) as sigpool,
            tc.tile_pool(name="outp", bufs=2) as outpool,
            # PSUM is 8 banks total and pool slots are static:
            # psA holds tags "sc0"/"sc1" ([128, GMAX*chunk] = 3 banks each, 6
            # total; projection/rot/Wo accumulators borrow these tags), psB
            # holds 2 PV accumulators (1 bank each). 6 + 2 = 8.
            tc.tile_pool(name="psA", bufs=1, space="PSUM") as psA,
            tc.tile_pool(name="psB", bufs=2, space="PSUM") as psB,
            tc.tile_pool(name="psC", bufs=2, space="PSUM") as psC,
        ):
            # ---- constants: weights (bf16), rotation matrix, cos/sin rows ----
            wq_sb = cpool.tile([128, dt8, DL], BF16, tag="wq")
            wk_sb = cpool.tile([128, dt8, DL], BF16, tag="wk")
            wv_sb = cpool.tile([128, dt8, DL], BF16, tag="wv")
            for wparam, wsb in ((wq, wq_sb), (wk, wk_sb), (wv, wv_sb)):
                wf = stpool.tile([128, dt8, DL], F32, tag="wstage")
                nc.sync.dma_start(wf[:], wparam.rearrange("(t p) m -> p t m", p=128))
                nc.scalar.copy(wsb[:], wf[:])

            mp_f = stpool.tile([DL, DL], F32, tag="mperm_f")
            nc.sync.dma_start(mp_f[:], mperm[:])
            mp_sb = cpool.tile([DL, DL], BF16, tag="mperm")
            nc.vector.tensor_copy(mp_sb[:], mp_f[:])

            id_f = stpool.tile([128, 128], F32, tag="ident_f")
            nc.sync.dma_start(id_f[:], ident[:])
            id_sb = cpool.tile([128, 128], BF16, tag="ident")
            nc.vector.tensor_copy(id_sb[:], id_f[:])

            # cos/sin: [s, 32] -> transposed, doubled rows -> [128, s] bf16
            st16 = s // 128
            cs128 = cpool.tile([128, s], BF16, tag="cs")
            sn128 = cpool.tile([128, s], BF16, tag="sn")
            for p, t128 in ((cosp, cs128), (sinp, sn128)):
                cf = stpool.tile([128, st16, HD // 2], F32, tag="cstage")
                nc.sync.dma_start(cf[:], p.rearrange("(t p) d -> p t d", p=128))
                cb = stpool.tile([128, st16, HD // 2], BF16, tag="cstage_b")
                nc.vector.tensor_copy(cb[:], cf[:])
                # XBAR transpose needs 128-divisible tiles: transpose 4
                # s-tiles (4*32 = 128 free) at once, then scatter row-blocks.
                for blk in range(st16 // 4):
                    ctmp = stpool.tile([128, 128], BF16, tag="cs_t")
                    nc.sync.dma_start_transpose(
                        out=ctmp[:], in_=cb[:, blk * 4 : (blk + 1) * 4, :]
                    )
                    for j in range(4):
                        st = blk * 4 + j
                        nc.sync.dma_start(
                            t128[0:32, st * 128 : (st + 1) * 128],
                            ctmp[j * 32 : (j + 1) * 32, :],
                        )
                for r in (32, 64, 96):
                    nc.sync.dma_start(t128[r : r + 32, :], t128[0:32, :])

            biasc = cpool.tile([128, 1], F32, tag="biasc")
            nc.vector.memset(biasc[:], EXP_BIAS)

            attnout = attpool.tile([DL, b * s], BF16, tag="attnout")
            att_sig = attpool.tile([HL, b * s], BF16, tag="att_sig")

            # Wo staging is off the startup critical path: emit after batch 0's
            # x pipeline in program order (scheduler fills DMA gaps with it).
            wo_sb = cpool.tile([128, dt8, D], BF16, tag="wo")

            for bi in range(b):
                # ---- x^T (bf16) via cast + DMA transpose ----
                xt_sb = xtpool.tile([128, dt8, s], BF16, tag="xt")
                for st in range(st16):
                    xf = xpool.tile([128, D], F32, tag="xf")
                    nc.sync.dma_start(xf[:], x[bi, st * 128 : (st + 1) * 128, :])
                    xb_ = xbpool.tile([128, D], BF16, tag="xb")
                    nc.vector.tensor_copy(xb_[:], xf[:])
                    # transpose via identity matmul: out = x_tile.T @ I.
                    # 4 transposes share one psum bank -> 1 batched copy.
                    for dt4 in range(dt8 // 4):
                        tps = psC.tile([128, 4, 128], F32, tag="tp")
                        for j in range(4):
                            dt = dt4 * 4 + j
                            nc.tensor.matmul(
                                tps[:, j, :],
                                xb_[:, dt * 128 : (dt + 1) * 128],
                                id_sb[:],
                                start=True,
                                stop=True,
                            )
                        nc.vector.tensor_copy(
                            xt_sb[:, dt4 * 4 : (dt4 + 1) * 4, st * 128 : (st + 1) * 128],
                            tps[:],
                        )

                # ---- q,k projections + RoPE; v projection + transpose ----
                q_rope = qkvpool.tile([DL, s], BF16, tag="q_rope")
                k_rope = qkvpool.tile([DL, s], BF16, tag="k_rope")
                vt_sb = qkvpool.tile([DL, s], BF16, tag="vt")
                for wsb, dst, is_v in (
                    (wq_sb, q_rope, False),
                    (wk_sb, k_rope, False),
                    (wv_sb, vt_sb, True),
                ):
                    for ch in range(nch):
                        cols = slice(ch * chunk, (ch + 1) * chunk)
                        ps = psA.tile([128, chunk], F32, tag=f"sc{ch % 2}")
                        for dt in range(dt8):
                            nc.tensor.matmul(
                                ps[:],
                                wsb[:, dt, :],
                                xt_sb[:, dt, cols],
                                start=(dt == 0),
                                stop=(dt == dt8 - 1),
                            )
                        if is_v:
                            nc.scalar.copy(dst[:, cols], ps[:])
                        else:
                            tsb = ropepool.tile([128, chunk], BF16, tag="tsb")
                            nc.scalar.copy(tsb[:], ps[:])
                            rps = psC.tile([128, chunk], F32, tag="tp")
                            nc.tensor.matmul(
                                rps[:], mp_sb[:], tsb[:], start=True, stop=True
                            )
                            m1 = ropepool.tile([128, chunk], BF16, tag="m1")
                            nc.vector.tensor_tensor(
                                m1[:], ps[:], cs128[:, cols], mybir.AluOpType.mult
                            )
                            m2 = ropepool.tile([128, chunk], BF16, tag="m2")
                            nc.vector.tensor_tensor(
                                m2[:], rps[:], sn128[:, cols], mybir.AluOpType.mult
                            )
                            nc.vector.tensor_tensor(
                                dst[:, cols], m1[:], m2[:], mybir.AluOpType.add
                            )

                if debug:
                    for name, tl in (("dbg_q", q_rope), ("dbg_k", k_rope), ("dbg_v", vt_sb)):
                        df = outpool.tile([DL, s], F32, tag="dbgf")
                        nc.vector.tensor_copy(df[:], tl[:])
                        nc.sync.dma_start(
                            {"dbg_q": dbg_q, "dbg_k": dbg_k, "dbg_v": dbg_v}[name][bi],
                            df[:],
                        )

                # v_aug [s_k, hd+1] blocks (ones column fuses the softmax sum)
                v_sb = qkvpool.tile([128, kt, HL, HD + 1], BF16, tag="v_sb")
                nc.vector.memset(v_sb[:, :, :, HD : HD + 1], 1.0)
                # transpose v^T -> v via identity matmul; the psum->sbuf
                # copies scatter the two head halves into the v_aug layout.
                for kt4 in range(kt // 4):
                    vps = psC.tile([128, 4, 128], F32, tag="tp")
                    for j in range(4):
                        ktt = kt4 * 4 + j
                        nc.tensor.matmul(
                            vps[:, j, :],
                            vt_sb[:, ktt * 128 : (ktt + 1) * 128],
                            id_sb[:],
                            start=True,
                            stop=True,
                        )
                    for h in range(HL):
                        nc.vector.tensor_copy(
                            v_sb[:, kt4 * 4 : (kt4 + 1) * 4, h, 0:HD],
                            vps[:, :, h * HD : (h + 1) * HD],
                        )

                if bi == 0:
                    # stage Wo now: overlaps batch-0 attention / batch-1 QKV
                    for dt in range(dt8):
                        wof = stpool.tile([128, D], F32, tag="wostage")
                        nc.sync.dma_start(wof[:], wo[dt * 128 : (dt + 1) * 128, :])
                        nc.scalar.copy(wo_sb[:, dt, :], wof[:])

                # ---- attention: transposed scores -> exp -> PV (+sigma) ----
                for ch in range(nch):
                    cols = slice(ch * chunk, (ch + 1) * chunk)
                    pts = {}
                    for gi, (k0, glen) in enumerate(groups):
                        for h in range(HL):
                            rows = slice(h * HD, (h + 1) * HD)
                            sg = psA.tile([128, GMAX, chunk], F32, tag=f"sc{h}")
                            for j in range(glen):
                                ktt = k0 + j
                                nc.tensor.matmul(
                                    sg[:, j, :],
                                    k_rope[rows, ktt * 128 : (ktt + 1) * 128],
                                    q_rope[rows, cols],
                                    start=True,
                                    stop=True,
                                )
                            pt = ptpool.tile([128, GMAX, chunk], BF16, tag="pt")
                            nc.scalar.activation(
                                pt[:, :glen, :],
                                sg[:, :glen, :],
                                mybir.ActivationFunctionType.Exp,
                                bias=biasc[:],
                                scale=EXP_SCALE,
                            )
                            pts[(gi, h)] = pt
                            if debug and bi == 0 and ch == 0 and gi == 0:
                                dsc = outpool.tile([128, chunk], F32, tag="dbgsc")
                                nc.vector.tensor_copy(dsc[:], sg[:, 0, :])
                                nc.sync.dma_start(dbg_sc[h], dsc[:])
                                dpt = outpool.tile([128, chunk], F32, tag="dbgpt")
                                nc.vector.tensor_copy(dpt[:], pt[:, 0, :])
                                nc.sync.dma_start(dbg_pt[h], dpt[:])
                    for h in range(HL):
                        pv = psB.tile([HD + 1, chunk], F32, tag="pv")
                        for gi, (k0, glen) in enumerate(groups):
                            pt = pts[(gi, h)]
                            for j in range(glen):
                                ktt = k0 + j
                                nc.tensor.matmul(
                                    pv[:],
                                    v_sb[:, ktt, h, :],
                                    pt[:, j, :],
                                    start=(ktt == 0),
                                    stop=(ktt == kt - 1),
                                )
                        if debug and bi == 0 and ch == 0:
                            dpv = outpool.tile([HD + 1, chunk], F32, tag="dbgpv")
                            nc.vector.tensor_copy(dpv[:], pv[:])
                            nc.sync.dma_start(dbg_pv[h], dpv[:])
                        # ship UNNORMALIZED numerator + sigma row; 1/sigma is
                        # applied once, consumer-side after the A2A
                        cols2 = slice(bi * s + ch * chunk, bi * s + (ch + 1) * chunk)
                        oh = sigpool.tile([HD + 1, chunk], BF16, tag="oh")
                        nc.vector.tensor_copy(oh[:], pv[:])
                        nc.gpsimd.dma_start(
                            attnout[h * HD : (h + 1) * HD, cols2], oh[0:HD, :]
                        )
                        nc.gpsimd.dma_start(
                            att_sig[h : h + 1, cols2], oh[HD : HD + 1, :]
                        )

            if debug:
                daf = outpool.tile([DL, b * s], F32, tag="dbga")
                nc.vector.tensor_copy(daf[:], attnout[:])
                nc.sync.dma_start(dbg_att[:], daf[:])

            # ---- AllToAll: attnout^T + sigma rows -> per-shard [1024, shard] ----
            a2a_in = dram.tile([N_CORES, DL + HL, shard], BF16, tag="a2a_in")
            a2a_out = dram.tile([N_CORES, DL + HL, shard], BF16, tag="a2a_out")
            for j in range(N_CORES):
                sl = slice(j * shard, (j + 1) * shard)
                nc.sync.dma_start(a2a_in[j, 0:DL, :], attnout[:, sl])
                nc.sync.dma_start(a2a_in[j, DL : DL + HL, :], att_sig[:, sl])
            nc.gpsimd.collective_compute(
                "AllToAll",
                mybir.AluOpType.bypass,
                replica_groups=[list(range(N_CORES))],
                ins=[a2a_in.opt()],
                outs=[a2a_out.opt()],
            )
            recv = cpool.tile([DL, N_CORES, shard], BF16, tag="recv")
            sigr = cpool.tile([N_CORES * HL, shard], BF16, tag="sigr")
            for i in range(N_CORES):
                nc.sync.dma_start(recv[:, i, :], a2a_out[i, 0:DL, :])
                nc.sync.dma_start(
                    sigr[i * HL : (i + 1) * HL, :], a2a_out[i, DL : DL + HL, :]
                )
            # one reciprocal for all 16 heads, per-half doubling broadcast,
            # one in-place scale of recv
            sigf = stpool.tile([N_CORES * HL, shard], F32, tag="sigf")
            nc.vector.tensor_copy(sigf[:], sigr[:])
            rf = stpool.tile([N_CORES * HL, shard], F32, tag="rf")
            nc.vector.reciprocal(rf[:], sigf[:])
            sigb = stpool.tile([N_CORES * HL, shard], BF16, tag="sigb")
            nc.vector.tensor_copy(sigb[:], rf[:])
            bca = attpool.tile([128, N_CORES, shard], BF16, tag="bca")
            for i in range(N_CORES):
                for h in range(HL):
                    nc.gpsimd.dma_start(
                        bca[h * HD : h * HD + 1, i, :],
                        sigb[i * HL + h : i * HL + h + 1, :],
                    )
            for h in range(HL):
                base = h * HD
                kk = 1
                while kk < HD:
                    nc.gpsimd.dma_start(
                        bca[base + kk : base + 2 * kk, :, :],
                        bca[base : base + kk, :, :],
                    )
                    kk *= 2
            nc.vector.tensor_tensor(
                recv[:], recv[:], bca[:], mybir.AluOpType.mult
            )

            # ---- output projection for this core's shard ----
            for j in range(shard // 128):
                for nco in range(D // 512):
                    wps = psA.tile([128, 512], F32, tag=f"sc{(j * 2 + nco) % 2}")
                    for i in range(N_CORES):
                        nc.tensor.matmul(
                            wps[:],
                            recv[:, i, j * 128 : (j + 1) * 128],
                            wo_sb[:, i, nco * 512 : (nco + 1) * 512],
                            start=(i == 0),
                            stop=(i == N_CORES - 1),
                        )
                    osb = outpool.tile([128, 512], F32, tag="osb")
                    nc.vector.tensor_copy(osb[:], wps[:])
                    nc.sync.dma_start(
                        out[j * 128 : (j + 1) * 128, nco * 512 : (nco + 1) * 512],
                        osb[:],
                    )

    split_excess_waits(nc)
    return nc


def make_in_maps(x, cos, sin, Wq, Wk, Wv, Wo, b, s):
    x = np.ascontiguousarray(x, dtype=np.float32)
    in_maps = []
    for c in range(N_CORES):
        cs = slice(c * DL, (c + 1) * DL)
        in_maps.append(
            {
                "x": x,
                "cos": np.ascontiguousarray(cos, dtype=np.float32),
                "sin": np.ascontiguousarray(sin, dtype=np.float32),
                "wq": np.ascontiguousarray(Wq[:, cs], dtype=np.float32),
                "wk": np.ascontiguousarray(Wk[:, cs], dtype=np.float32),
                "wv": np.ascontiguousarray(Wv[:, cs], dtype=np.float32),
                "wo": np.ascontiguousarray(Wo, dtype=np.float32),
            }
        )
    return in_maps


_NC_CACHE = {}


def run(x, cos, sin, Wq, Wk, Wv, Wo, trace=False, chunk=512, pt_bufs=16):
    b, s, _ = x.shape
    key = (b, s, chunk, pt_bufs)
    if key not in _NC_CACHE:
        _NC_CACHE[key] = build_nc(b=b, s=s, chunk=chunk, pt_bufs=pt_bufs)
    nc = _NC_CACHE[key]
    in_maps = make_in_maps(x, cos, sin, Wq, Wk, Wv, Wo, b, s)
    res = run_bass_kernel_spmd(nc, in_maps, list(range(N_CORES)), trace=trace)
    shard = b * s // N_CORES
    full = np.concatenate([res.results[c]["out"] for c in range(N_CORES)], axis=0)
    return full.reshape(b, s, D), res


def kernel(x, cos, sin, Wq, Wk, Wv, Wo):
    out, _ = run(
        np.asarray(x), np.asarray(cos), np.asarray(sin),
        np.asarray(Wq), np.asarray(Wk), np.asarray(Wv), np.asarray(Wo),
    )
    return out.astype(np.float32)
